# revision 44
# baseline (speedup 1.0000x reference)
"""Trainium2 Bass kernel for a dense transformer DecoderLayer.

Layer: x = q
  x += SelfAttn(LN1(x))   (causal, 8 heads)
  x += CrossAttn(LN2(x), k, v)
  x += FFN(LN3(x))        (E -> 4E relu -> E)

Sharding: 8 cores = (batch b = core//2, parity p = core%2). Core (b, p)
owns the 8 odd-or-even 128-row stripes of batch b's 2048 query rows.
Host permutes q[b]^T columns to [partner stripes | own stripes] so the
device program is identical on every core (SPMD); the parity-dependent
causal boundary is carried by a data mask (mp = all-ones or all-zeros).

Device dataflow is fully "transposed": the residual stream lives as
x^T [E=512 partitions(4 tiles), tokens] so no on-device activation
transposes are needed except the final 128x128 PE transposes on output.
Scores are computed transposed (S^T [s,q]) so the softmax denominator
falls out of the P@V matmul via an appended ones-column on V.
"""

import numpy as np
import ml_dtypes

import concourse.bass as bass
import concourse.tile as tile
from concourse import bacc
from concourse import mybir
from concourse.bass_utils import run_bass_kernel_spmd

F32 = mybir.dt.float32
F32R = mybir.dt.float32r
BF16 = mybir.dt.bfloat16
FP8 = mybir.dt.float8e4

B, T_FULL, E, H, D, FW = 4, 2048, 512, 8, 64, 4
EC = E // 128           # e-chunks
F = FW * E              # ffn hidden
FC = F // 128
EPS = 1e-5
NCORES = 8


def _pieces(a, b, step=512):
    """Split [a, b) at multiples of `step` (PSUM-bank aligned pieces)."""
    out = []
    while a < b:
        nxt = min(b, (a // step + 1) * step)
        out.append((a, nxt))
        a = nxt
    return out


def _pieces_bf(a, b):
    """Matmul output pieces: one PSUM bank (512 f32) per matmul — walrus
    rejects bank-spanning matmul outputs."""
    return _pieces(a, b, 512)


def build_nc(T=T_FULL):
    R = T // 2           # own query columns (packed at [R:T])
    G = R // 128         # own 128-col groups
    NCH = T // 128       # total s-chunks

    nc = bacc.Bacc(None, target_bir_lowering=False)

    # ---------------- DRAM I/O ----------------
    qTp = nc.dram_tensor("qTp", [E, T], F32, kind="ExternalInput")
    kT = nc.dram_tensor("kT", [E, T], mybir.dt.float8e4, kind="ExternalInput")
    vT = nc.dram_tensor("vT", [E, T], mybir.dt.float8e4, kind="ExternalInput")
    w = {}
    for nm in ("wq_s", "wk_s", "wv_s", "wq_c", "wk_c", "wv_c"):
        w[nm] = nc.dram_tensor(nm, [E, H * D], mybir.dt.float8e4, kind="ExternalInput")
    w["wp_s"] = nc.dram_tensor("wp_s", [H * D, E], BF16, kind="ExternalInput")
    w["wp_c"] = nc.dram_tensor("wp_c", [H * D, E], BF16, kind="ExternalInput")
    w["w1"] = nc.dram_tensor("w1", [E, F], mybir.dt.float8e4, kind="ExternalInput")
    w["w2"] = nc.dram_tensor("w2", [F, E], mybir.dt.float8e4, kind="ExternalInput")
    bias_d = {}
    for nm, sz in (("bq_s", H * D), ("bk_s", H * D), ("bq_c", H * D), ("bk_c", H * D),
                   ("bp_s", E), ("bp_c", E), ("b1f", F)):
        bias_d[nm] = nc.dram_tensor(nm, [sz], F32, kind="ExternalInput")
    msk2_d = nc.dram_tensor("msk2", [128, 256], BF16, kind="ExternalInput")
    ident_d = nc.dram_tensor("ident", [128, 128], F32, kind="ExternalInput")
    out_d = nc.dram_tensor("out", [R, E], F32, kind="ExternalOutput")

    with tile.TileContext(nc) as tc:
        with (
            tc.tile_pool(name="resident", bufs=1) as res,
            tc.tile_pool(name="consts", bufs=1) as cpool,
            tc.tile_pool(name="work", bufs=2) as work,
            tc.tile_pool(name="es_pool", bufs=3) as es_pool,
            tc.tile_pool(name="stat", bufs=2) as stat,
            tc.tile_pool(name="drampool", bufs=2, space="DRAM") as drampool,
            tc.tile_pool(name="psum", bufs=1, space="PSUM") as psum,
        ):
            # ---- PSUM tags: "sc" 2bk x2, "o" 1bk x2, "st" 1bk x2 = 8 banks
            def ps_sc(name):
                """[128, 2, 512] scores-pair tile; projections use [:, 0, :]."""
                return psum.tile([128, 2, 512], F32, name=name, tag="sc", bufs=2)

            def ps_o(name, shape=None):
                return psum.tile(shape or [128, 512], F32, name=name, tag="o", bufs=3)

            # ---------------- resident loads ----------------
            xto = []        # own-half residual stream [E, R], lives whole kernel
            for c in range(EC):
                t_ = res.tile([128, R], F32, name=f"xto{c}")
                nc.sync.dma_start(
                    out=t_, in_=qTp.rearrange("(c p) t -> c p t", p=128)[c][:, R:T])
                xto.append(t_)

            bias_sb = {}
            for nm in bias_d:
                src = bias_d[nm]
                t_ = cpool.tile([128, src.shape[0] // 128], F32, name=f"b_{nm}")
                nc.sync.dma_start(out=t_, in_=src.rearrange("(c p) -> p c", p=128))
                bias_sb[nm] = t_
            msk2_sb = cpool.tile([128, 2, 128], BF16, name="msk2_sb")
            nc.sync.dma_start(out=msk2_sb, in_=msk2_d[:, :])
            ident_sb = cpool.tile([128, 128], F32, name="ident_sb")
            nc.sync.dma_start(out=ident_sb, in_=ident_d[:, :])
            # fp8 ones/E for DoubleRow LN-stats lhsT (middle stride 16: ok)
            ones8 = cpool.tile([128, 2, 16], FP8, name="ones8")
            nc.vector.memset(ones8, 1.0 / E)
            eps_sb = cpool.tile([1, 1], F32, name="eps_sb")
            nc.vector.memset(eps_sb, EPS)

            def load_w(pool, nm, tag):
                src = w[nm]
                if nm.startswith("wp"):
                    # head-pair packed: rows [hp*128:(hp+1)*128] = heads 2hp,2hp+1
                    t_ = pool.tile([128, H // 2, E], BF16, name=f"sb_{nm}", tag=tag)
                    nc.sync.dma_start(out=t_, in_=src.rearrange("(hp p) n -> p hp n", p=128))
                else:
                    t_ = pool.tile([128, src.shape[0] // 128, src.shape[1]],
                                   src.dtype, name=f"sb_{nm}", tag=tag)
                    nc.sync.dma_start(out=t_, in_=src.rearrange("(c p) n -> p c n", p=128))
                return t_

            # ---------------- transposed layernorm (per 512-token piece) ----
            def ln_half(xap, xn8, tag, qa, qb, prof):
                """LN over token cols [qa, qb) (<=512 wide). xap(c, a, b) ->
                [128, b-a] f32 SBUF AP; writes normalized fp8 to
                xn8[:, c, qa:qb]. prof picks engines: "pre" = Act square
                (idle prefix), "mid" = Pool (Act busy with exp)."""
                wd = qb - qa
                xs = work.tile([128, EC, 512], FP8, name=f"xs_{tag}", tag="lnxs", bufs=2)
                sqt = work.tile([128, EC, 512], FP8, name=f"sq_{tag}", tag="lnsq", bufs=2)
                for c in range(EC):
                    xa = xap(c, qa, qb)
                    if prof == "pre":
                        nc.scalar.activation(sqt[:, c, :wd], xa,
                                             mybir.ActivationFunctionType.Square)
                    else:
                        nc.gpsimd.tensor_mul(sqt[:, c, :wd], xa, xa)
                    nc.gpsimd.tensor_copy(xs[:, c, :wd], xa)
                stp = ps_sc(f"st_{tag}")     # stats ride the sc tag's 2 banks
                st0, st1 = stp[0:1, 0, :], stp[0:1, 1, :]
                for c in range(0, EC, 2):
                    nc.tensor.matmul(st0[:, :wd], ones8[:, :, 0:1], xs[:, c:c + 2, :wd],
                                     start=(c == 0), stop=(c == EC - 2),
                                     perf_mode=mybir.MatmulPerfMode.DoubleRow,
                                     skip_group_check=True)
                    nc.tensor.matmul(st1[:, :wd], ones8[:, :, 0:1], sqt[:, c:c + 2, :wd],
                                     start=(c == 0), stop=(c == EC - 2),
                                     perf_mode=mybir.MatmulPerfMode.DoubleRow,
                                     skip_group_check=True)
                var = stat.tile([1, 512], F32, name=f"var_{tag}", tag="var", bufs=2)
                m_sb = stat.tile([1, 512], F32, name=f"m_{tag}", tag="m_sb", bufs=2)
                nc.vector.tensor_copy(m_sb[:, :wd], st0[:, :wd])
                nc.vector.tensor_mul(var[:, :wd], m_sb[:, :wd], st0[:, :wd])
                nc.vector.tensor_sub(var[:, :wd], st1[:, :wd], var[:, :wd])
                # rsqrt = exp(-0.5*ln(var+eps)): stays in the ln/exp act table
                nc.scalar.activation(var[:, :wd], var[:, :wd],
                                     mybir.ActivationFunctionType.Ln,
                                     bias=eps_sb[0:1, 0:1])
                nc.scalar.activation(var[:, :wd], var[:, :wd],
                                     mybir.ActivationFunctionType.Exp, scale=-0.5)
                mb = work.tile([128, 512], F32, name=f"mb_{tag}", tag="mb", bufs=2)
                rb = work.tile([128, 512], F32, name=f"rb_{tag}", tag="rb", bufs=2)
                m_dr = drampool.tile([1, 512], F32, name=f"mdr_{tag}", tag="mdr", bufs=3)
                r_dr = drampool.tile([1, 512], F32, name=f"rdr_{tag}", tag="rdr", bufs=3)
                nc.sync.dma_start(out=m_dr[:, :wd], in_=m_sb[:, :wd])
                nc.sync.dma_start(out=r_dr[:, :wd], in_=var[:, :wd])
                nc.sync.dma_start(out=mb[:, :wd], in_=m_dr[:, :wd].to_broadcast((128, wd)))
                nc.sync.dma_start(out=rb[:, :wd], in_=r_dr[:, :wd].to_broadcast((128, wd)))
                for c in range(EC):
                    tmp = work.tile([128, 512], F32, name=f"lt_{tag}", tag="lntmp", bufs=2)
                    nc.vector.tensor_sub(tmp[:, :wd], xap(c, qa, qb), mb[:, :wd])
                    nc.vector.tensor_mul(xn8[:, c, qa:qb], tmp[:, :wd], rb[:, :wd])

            # ---------------- attention building blocks ----------------
            def evac_proj(out_ap, ps_ap, b_ap, eng):
                """PSUM -> SBUF evac: out = ps/16 + bias, on chosen engine."""
                if eng == "act":
                    nc.scalar.activation(out_ap, ps_ap,
                                         mybir.ActivationFunctionType.Identity,
                                         bias=b_ap, scale=1.0 / 16.0)
                else:
                    nc.vector.tensor_scalar(out_ap, ps_ap, 1.0 / 16.0, b_ap,
                                            op0=mybir.AluOpType.mult,
                                            op1=mybir.AluOpType.add)

            def proj_kt(apool, tags, src_aps, wk_t, bk_sb, tag, order=None):
                """K^T head-pair tiles [128, T]; emits all pieces (prefix)."""
                ktp = [apool.tile([128, T], BF16, name=f"ktp_{tag}{pp}", tag=tags[pp])
                       for pp in range(4)]
                pcs = order or _pieces_bf(0, T)
                for (a, b_) in pcs:
                    for pp in range(4):
                        ps = ps_sc(f"kps_{tag}")
                        for ci, c in enumerate(range(0, EC, 2)):
                            nc.tensor.matmul(
                                ps[:, 0, :b_ - a],
                                wk_t[:, c:c + 2, pp * 128:(pp + 1) * 128],
                                src_aps[ci](a, b_),
                                start=(c == 0), stop=(c == EC - 2),
                                perf_mode=mybir.MatmulPerfMode.DoubleRow)
                        evac_proj(ktp[pp][:, a:b_], ps[:, 0, :b_ - a],
                                  bk_sb[:, pp:pp + 1], "act" if pp % 2 == 0 else "dve")
                return ktp

            def proj_qt_half(qtp, xn_q, wq_t, bq_sb, tag, qa, qb, eng, pp=None):
                """Q^T projection for token cols [qa, qb). pp=None -> all 4."""
                for p_ in ([pp] if pp is not None else range(4)):
                    ps = ps_sc(f"qps_{tag}")
                    for ci, c in enumerate(range(0, EC, 2)):
                        nc.tensor.matmul(
                            ps[:, 0, :qb - qa],
                            wq_t[:, c:c + 2, p_ * 128:(p_ + 1) * 128],
                            xn_q[ci](qa, qb),
                            start=(c == 0), stop=(c == EC - 2),
                            perf_mode=mybir.MatmulPerfMode.DoubleRow)
                    e_ = ("act" if p_ % 2 == 0 else "dve") if eng == "mix0" else eng
                    evac_proj(qtp[p_][:, qa:qb], ps[:, 0, :qb - qa],
                              bq_sb[:, p_:p_ + 1], e_)

            def heads_half(ktp, qtp, v_pair, causal, tag, o_out, qa, qb,
                           fillers=None, heads=None):
                """One q-half (cols [qa, qb)) of all 8 heads. v_pair(pr, h) ->
                lhsT AP [128, 2, D+2] fp8.  Scores for a chunk pair share one
                [128, 2, 512] PSUM tile -> one exp -> fp8 es -> DoubleRow P@V.
                fillers[h] = callables emitted after head h (fill idle engines
                while Act grinds exp)."""
                VP = D + 2
                wd0 = qb - qa
                for h in (heads if heads is not None else range(H)):
                    pp, hr = h // 2, (h % 2) * 64
                    o_ps = ps_o(f"ops_{tag}", [VP, 512])
                    prs = [g for g in range(G) if (not causal) or g * 128 < qb]
                    for pi, pr in enumerate(prs):
                        if causal:
                            kA, kB, q0 = pr, G + pr, pr * 128
                        else:
                            kA, kB, q0 = 2 * pr, 2 * pr + 1, 0
                        a = max(q0, qa)
                        wd = qb - a
                        sc = ps_sc(f"scps_{tag}")
                        nc.tensor.matmul(
                            sc[:, 0, :wd],
                            ktp[pp][hr:hr + 64, kA * 128:(kA + 1) * 128],
                            qtp[pp][hr:hr + 64, a:qb],
                            start=True, stop=True)
                        nc.tensor.matmul(
                            sc[:, 1, :wd],
                            ktp[pp][hr:hr + 64, kB * 128:(kB + 1) * 128],
                            qtp[pp][hr:hr + 64, a:qb],
                            start=True, stop=True)
                        es = es_pool.tile([128, 2, 512], FP8,
                                          name=f"es_{tag}", tag="es")
                        nc.scalar.activation(es[:, :, :wd], sc[:, :, :wd],
                                             mybir.ActivationFunctionType.Exp)
                        if causal and q0 >= qa:
                            # diagonal block: [mp | tri] masks in one Pool op
                            nc.gpsimd.tensor_mul(es[:, :, 0:128],
                                                 es[:, :, 0:128], msk2_sb)
                        nc.tensor.matmul(
                            o_ps[:, a - qa:wd0], v_pair(pr, h), es[:, :, :wd],
                            start=(pi == 0), stop=(pi == len(prs) - 1),
                            perf_mode=mybir.MatmulPerfMode.DoubleRow,
                            skip_group_check=True)
                    dn = stat.tile([1, 512], F32, name=f"dn_{tag}", tag="dn")
                    nc.vector.reciprocal(dn[:, :wd0], o_ps[D:D + 1, :wd0])
                    rb_h = work.tile([64, 512], F32, name=f"rbh_{tag}", tag="rbh", bufs=2)
                    nc.gpsimd.partition_broadcast(rb_h[:, :wd0], dn[:, :wd0])
                    nc.vector.tensor_mul(o_out[pp][hr:hr + 64, qa:qb],
                                         o_ps[0:D, :wd0], rb_h[:, :wd0])
                    if fillers and h in fillers:
                        for f_ in fillers[h]:
                            f_()

            def out_proj_eb(o_pairs, wp_t, bp_sb, tag, qa, qb, eb):
                ps = ps_o(f"yps_{tag}")
                for j in range(H // 2):
                    nc.tensor.matmul(
                        ps[:, :qb - qa],
                        wp_t[:, j, eb * 128:(eb + 1) * 128],
                        o_pairs[j][:, qa:qb],
                        start=(j == 0), stop=(j == H // 2 - 1))
                nc.vector.scalar_tensor_tensor(
                    xto[eb][:, qa:qb], ps[:, :qb - qa], bp_sb[:, eb:eb + 1],
                    xto[eb][:, qa:qb],
                    op0=mybir.AluOpType.add, op1=mybir.AluOpType.add)

            # ================ forward ================
            with tc.tile_pool(name="w_attn", bufs=1) as wat, \
                 tc.tile_pool(name="p_self", bufs=1) as pself:
                # partner-half of residual input (dies after LN1)
                xtp = []
                for c in range(EC):
                    t_ = pself.tile([128, R], F32, name=f"xtp{c}", tag=f"xtp{c}")
                    nc.sync.dma_start(
                        out=t_, in_=qTp.rearrange("(c p) t -> c p t", p=128)[c][:, 0:R])
                    xtp.append(t_)

                wq_t = load_w(wat, "wq_s", "wq")
                wk_t = load_w(wat, "wk_s", "wk")
                wv_t = load_w(wat, "wv_s", "wv")

                def xap1(c, a, b_):
                    if b_ <= R:
                        return xtp[c][:, a:b_]
                    return xto[c][:, a - R:b_ - R]

                # LN1: emit only the pieces heads-h0 needs, defer the rest
                xn1 = pself.tile([128, EC, T], FP8, name="xn1")
                ln_half(xap1, xn1, "ln1", 0, 512, "pre")
                ln_half(xap1, xn1, "ln1", 1024, 1536, "pre")

                # --- self-attn projections: first-need pieces, rest deferred
                ktp_s = [pself.tile([128, T], BF16, name=f"ktp_s{pp}",
                                    tag=f"ktp{pp}") for pp in range(4)]
                xn_k1 = [lambda a, b_, c=c: xn1[:, c:c + 2, a:b_]
                         for c in range(0, EC, 2)]

                def k_step_s(a, b_, pp, eng="dve"):
                    def run():
                        ps = ps_sc("kps_sa")
                        for ci, c in enumerate(range(0, EC, 2)):
                            nc.tensor.matmul(
                                ps[:, 0, :b_ - a],
                                wk_t[:, c:c + 2, pp * 128:(pp + 1) * 128],
                                xn_k1[ci](a, b_),
                                start=(c == 0), stop=(c == EC - 2),
                                perf_mode=mybir.MatmulPerfMode.DoubleRow)
                        evac_proj(ktp_s[pp][:, a:b_], ps[:, 0, :b_ - a],
                                  bias_sb["bk_s"][:, pp:pp + 1], eng)
                    return run

                for (a, b_) in ((0, 512), (1024, 1536)):
                    for pp in range(4):
                        k_step_s(a, b_, pp, "act" if pp % 2 == 0 else "dve")()
                qtp_s = [pself.tile([128, R], BF16, name=f"qtp_s{pp}", tag=f"qtp{pp}")
                         for pp in range(4)]
                xn_q1 = [lambda a, b_, c=c: xn1[:, c:c + 2, R + a:R + b_]
                         for c in range(0, EC, 2)]
                proj_qt_half(qtp_s, xn_q1, wq_t, bias_sb["bq_s"], "sa", 0, 512, "mix0")
                # V chunk-pair tiles [128, 2, H, D+2] fp8: slot 0 = partner
                # stripe g (chunk g), slot 1 = own stripe g (chunk G+g)
                v_sb_s = [pself.tile([128, 2, H, D + 2], FP8, name=f"vpr_sa{g}",
                                     tag=f"vsb{g}") for g in range(G)]

                def v_step_s(k, eng):
                    def run():
                        pair, slot = (k, 0) if k < G else (k - G, 1)
                        ps = ps_sc("vps_sa")
                        for c in range(0, EC, 2):
                            nc.tensor.matmul(
                                ps[:, 0, :512],
                                xn1[:, c:c + 2, k * 128:(k + 1) * 128],
                                wv_t[:, c:c + 2, :],
                                start=(c == 0), stop=(c == EC - 2),
                                perf_mode=mybir.MatmulPerfMode.DoubleRow)
                        vs = v_sb_s[pair]
                        if eng == "act":
                            nc.scalar.activation(
                                vs[:, slot, :, 0:D],
                                ps[:, 0, :512].rearrange("p (h d) -> p h d", h=H),
                                mybir.ActivationFunctionType.Copy, scale=1.0 / 16.0)
                        else:
                            nc.vector.tensor_scalar(
                                vs[:, slot, :, 0:D],
                                ps[:, 0, :512].rearrange("p (h d) -> p h d", h=H),
                                1.0 / 16.0, None, op0=mybir.AluOpType.mult)
                        nc.gpsimd.memset(vs[:, slot, :, D:D + 2], 1.0)
                    return run

                for k in list(range(0, 4)) + list(range(8, 12)):
                    v_step_s(k, "act" if k % 2 == 0 else "dve")()

                # cross/FFN weights: DMA-queued here so they don't delay
                # the LN1 broadcasts, but still arrive well before first use
                wp_t = load_w(wat, "wp_s", "wp")
                wk_ct = load_w(wat, "wk_c", "wk_c")
                wv_ct = load_w(wat, "wv_c", "wv_c")
                wq_ct = load_w(wat, "wq_c", "wq_c")
                wp_ct = load_w(wat, "wp_c", "wp_c")
                w1_t = load_w(wat, "w1", "w1")
                w2_t = load_w(wat, "w2", "w2")

                # --- cross K/V projection steps (fillers) ---
                ktp_c = [pself.tile([128, T], BF16, name=f"ktp_c{pp}",
                                    tag=f"xn1{'abcd'[pp]}") for pp in range(4)]
                kcache = {}

                def ck_step(a, b_, pp):
                    def run():
                        if a not in kcache:
                            t_ = pself.tile([128, EC, 512], FP8, name="ksl_ca",
                                            tag=f"xtp{(a // 512) % 4}", bufs=1)
                            nc.sync.dma_start(
                                out=t_[:, :, :b_ - a],
                                in_=kT.rearrange("(c p) t -> p c t", p=128)[:, :, a:b_])
                            kcache[a] = t_
                        ps = ps_sc("kps_ca")
                        for c in range(0, EC, 2):
                            nc.tensor.matmul(
                                ps[:, 0, :b_ - a],
                                wk_ct[:, c:c + 2, pp * 128:(pp + 1) * 128],
                                kcache[a][:, c:c + 2, :b_ - a],
                                start=(c == 0), stop=(c == EC - 2),
                                perf_mode=mybir.MatmulPerfMode.DoubleRow)
                        evac_proj(ktp_c[pp][:, a:b_], ps[:, 0, :b_ - a],
                                  bias_sb["bk_c"][:, pp:pp + 1], "dve")
                    return run

                v_g = [pself.tile([128, 8, H, D + 2], FP8, name=f"vg{g2}",
                                  tag=f"vg{g2}") for g2 in range(2)]
                vcache = {}

                def cv_step(k):
                    def run():
                        k4 = (k // 4) * 4
                        if k4 not in vcache:
                            vsl = work.tile([128, EC, 512], FP8, name="vsl_ca",
                                            tag="vsl", bufs=2)
                            nc.sync.dma_start(
                                out=vsl,
                                in_=vT.rearrange("(c p) t -> p c t", p=128)
                                [:, :, k4 * 128:(k4 + 4) * 128])
                            vcache[k4] = vsl
                        off = (k - k4) * 128
                        ps = ps_sc("vps_ca")
                        for c in range(0, EC, 2):
                            nc.tensor.matmul(
                                ps[:, 0, :512],
                                vcache[k4][:, c:c + 2, off:off + 128],
                                wv_ct[:, c:c + 2, :],
                                start=(c == 0), stop=(c == EC - 2),
                                perf_mode=mybir.MatmulPerfMode.DoubleRow)
                        nc.vector.tensor_scalar(
                            v_g[k // 8][:, k % 8, :, 0:D],
                            ps[:, 0, :512].rearrange("p (h d) -> p h d", h=H),
                            1.0 / 16.0, None, op0=mybir.AluOpType.mult)
                        nc.gpsimd.memset(v_g[k // 8][:, k % 8, :, D:D + 2], 1.0)
                    return run

                def gen_of(steps):
                    for s_ in steps:
                        s_()
                        yield

                def take(gen, n):
                    def run():
                        for _ in range(n):
                            try:
                                next(gen)
                            except StopIteration:
                                break
                    return run

                # deferred prefix work, ordered so heads-h1 inputs finish first
                rest_steps = [lambda: ln_half(xap1, xn1, "ln1", 512, 1024, "mid")]
                rest_steps += [k_step_s(512, 1024, pp) for pp in range(4)]
                rest_steps.append(lambda: ln_half(xap1, xn1, "ln1", 1536, 2048, "mid"))
                rest_steps += [k_step_s(1536, 2048, pp) for pp in range(4)]
                rest_steps += [lambda pp=pp: proj_qt_half(
                    qtp_s, xn_q1, wq_t, bias_sb["bq_s"], "sa", 512, 1024, "dve", pp)
                    for pp in range(4)]
                rest_steps += [v_step_s(k, "dve")
                               for k in list(range(4, 8)) + list(range(12, 16))]
                rsg = gen_of(rest_steps)
                ckg = gen_of([ck_step(a, b_, pp)
                              for (a, b_) in _pieces_bf(0, T) for pp in range(4)])
                cvg = gen_of([cv_step(k) for k in range(NCH)])

                # --- self attention, q-half pipelined ---
                o1 = [pself.tile([128, R], BF16, name=f"opr1{j}", tag=f"opr{j}")
                      for j in range(H // 2)]
                v_at1 = lambda pr, h: v_sb_s[pr][:, :, h, :]
                fill0 = {0: [take(rsg, 1)], 1: [take(rsg, 4)], 2: [take(rsg, 5)],
                         3: [take(rsg, 4)], 4: [take(rsg, 4)], 5: [take(rsg, 4)],
                         6: [take(rsg, 2), take(ckg, 2)],
                         7: [take(ckg, 3), take(cvg, 3)]}
                heads_half(ktp_s, qtp_s, v_at1, True, "sa", o1, 0, 512, fill0)
                take(rsg, 99)()

                # tail of half0 (outproj + LN2 + Qc) woven into half1
                xn2 = pself.tile([128, EC, R], FP8, name="xn2")
                qtp_ct = [pself.tile([128, R], BF16, name=f"qtp_c{pp}",
                                     tag=f"qtpc{pp}") for pp in range(4)]
                xap2 = lambda c, a, b_: xto[c][:, a:b_]
                xn_q2 = [lambda a, b_, c=c: xn2[:, c:c + 2, a:b_]
                         for c in range(0, EC, 2)]
                t0_steps = [lambda eb=eb: out_proj_eb(o1, wp_t, bias_sb["bp_s"],
                                                      "sa", 0, 512, eb)
                            for eb in range(EC)]
                t0_steps.append(lambda: ln_half(xap2, xn2, "ln2a", 0, 512, "mid"))
                t0_steps += [lambda pp=pp: proj_qt_half(
                    qtp_ct, xn_q2, wq_ct, bias_sb["bq_c"], "ca", 0, 512, "dve", pp)
                    for pp in range(4)]
                t0g = gen_of(t0_steps)
                # o2 aliases qtp_s tags: qtp_s[j] is last read by self-h1 head
                # 2j+1, well before cross-h0 head 2j writes o2[j]
                o2 = [pself.tile([128, R], BF16, name=f"opr2{j}", tag=f"qtp{j}")
                      for j in range(H // 2)]
                v_at2 = lambda pr, h: v_g[(2 * pr) // 8][:, (2 * pr) % 8:(2 * pr) % 8 + 2, h, :]

                def xh0(heads):
                    return lambda: heads_half(ktp_c, qtp_ct, v_at2, False, "ca",
                                              o2, 0, 512, None, heads=heads)

                fill1 = {0: [take(ckg, 6), take(cvg, 3)],
                         1: [take(ckg, 5), take(cvg, 3)],
                         2: [take(cvg, 3), take(t0g, 2)],
                         3: [take(cvg, 4), take(t0g, 2)],
                         4: [take(t0g, 1)], 5: [take(t0g, 2)],
                         6: [take(t0g, 2), xh0([0, 1])],
                         7: [xh0([2, 3])]}
                heads_half(ktp_s, qtp_s, v_at1, True, "sa", o1, 512, 1024, fill1)
                take(ckg, 99)(); take(cvg, 99)(); take(t0g, 99)()

                # half1 tail: out-proj, then rest of cross-h0 with LN2-h1/Qc-h1
                for eb in range(EC):
                    out_proj_eb(o1, wp_t, bias_sb["bp_s"], "sa", 512, 1024, eb)
                tb_steps = [lambda: ln_half(xap2, xn2, "ln2b", 512, 1024, "mid")]
                tb_steps += [lambda pp=pp: proj_qt_half(
                    qtp_ct, xn_q2, wq_ct, bias_sb["bq_c"], "ca", 512, 1024, "dve", pp)
                    for pp in range(4)]
                tbg = gen_of(tb_steps)
                fillc0 = {4: [take(tbg, 1)], 5: [take(tbg, 2)], 6: [take(tbg, 2)]}
                heads_half(ktp_c, qtp_ct, v_at2, False, "ca", o2, 0, 512, fillc0,
                           heads=[4, 5, 6, 7])
                take(tbg, 99)()

                # tail of cross-h0 (outproj + LN3 + FFN-h0) woven into cross-h1
                # tag-alias onto tiles whose last readers are already done:
                # xn3 reuses xn2's space (dead after qtp_c), h1t reuses xn1's
                xn3 = pself.tile([128, EC, R], FP8, name="xn3", tag="xn2")
                h1t = pself.tile([128, FC, R], FP8, name="h1t", tag="xn1")

                def ffn1_f(qa, qb, f, eng):
                    # h0 fillers use the stats bank; the h1 tail can take the
                    # scores tag (attention is finished there)
                    if qa == 0:
                        ps = psum.tile([128, 512], F32, name="hps", tag="st", bufs=1)
                    else:
                        ps = ps_sc("hps")[:, 0, :]
                    for c in range(0, EC, 2):
                        nc.tensor.matmul(
                            ps[:, :qb - qa],
                            w1_t[:, c:c + 2, f * 128:(f + 1) * 128],
                            xn3[:, c:c + 2, qa:qb],
                            start=(c == 0), stop=(c == EC - 2),
                            perf_mode=mybir.MatmulPerfMode.DoubleRow)
                    if eng == "act":
                        nc.scalar.activation(
                            h1t[:, f, qa:qb], ps[:, :qb - qa],
                            mybir.ActivationFunctionType.Relu,
                            bias=bias_sb["b1f"][:, f:f + 1])
                    else:
                        nc.vector.tensor_scalar(
                            h1t[:, f, qa:qb], ps[:, :qb - qa],
                            bias_sb["b1f"][:, f:f + 1], 0.0,
                            op0=mybir.AluOpType.add, op1=mybir.AluOpType.max)

                def ffn2_eb(qa, qb, eb):
                    ps = ps_o("y2ps")
                    for f in range(0, FC, 2):
                        nc.tensor.matmul(
                            ps[:, :qb - qa],
                            w2_t[:, f:f + 2, eb * 128:(eb + 1) * 128],
                            h1t[:, f:f + 2, qa:qb],
                            start=(f == 0), stop=(f == FC - 2),
                            perf_mode=mybir.MatmulPerfMode.DoubleRow)
                    nc.vector.scalar_tensor_tensor(
                        xto[eb][:, qa:qb], ps[:, :qb - qa], 1.0 / 256.0,
                        xto[eb][:, qa:qb],
                        op0=mybir.AluOpType.mult, op1=mybir.AluOpType.add)

                tc0_steps = [lambda eb=eb: out_proj_eb(o2, wp_ct, bias_sb["bp_c"],
                                                       "ca", 0, 512, eb)
                             for eb in range(EC)]
                tc0_steps.append(lambda: ln_half(xap2, xn3, "ln3a", 0, 512, "mid"))
                tc0_steps += [lambda f=f: ffn1_f(0, 512, f, "dve")
                              for f in range(FC)]
                tc0_steps += [lambda eb=eb: ffn2_eb(0, 512, eb) for eb in range(EC)]
                tcg = gen_of(tc0_steps)
                fillc = {0: [take(tcg, 2)], 1: [take(tcg, 3)], 2: [take(tcg, 3)],
                         3: [take(tcg, 3)], 4: [take(tcg, 3)], 5: [take(tcg, 3)],
                         6: [take(tcg, 4)], 7: [take(tcg, 4)]}
                heads_half(ktp_c, qtp_ct, v_at2, False, "ca", o2, 512, 1024, fillc)
                take(tcg, 99)()

                # cross half1 tail + FFN half1 + transpose/store per eb
                for eb in range(EC):
                    out_proj_eb(o2, wp_ct, bias_sb["bp_c"], "ca", 512, 1024, eb)
                ln_half(xap2, xn3, "ln3b", 512, 1024, "pre")
                for f in range(FC):
                    ffn1_f(512, 1024, f, "act" if f % 2 == 0 else "dve")
                for eb in range(EC):
                    ffn2_eb(512, 1024, eb)
                    for tb in range(R // 128):
                        ps = psum.tile([128, 128], F32, name="trp", tag="o", bufs=3)
                        nc.tensor.transpose(
                            ps, xto[eb][:, tb * 128:(tb + 1) * 128], ident_sb)
                        ott = work.tile([128, 128], F32, name="ott", tag="ott", bufs=6)
                        if tb % 2 == 0:
                            nc.vector.tensor_copy(ott, ps)
                        else:
                            nc.scalar.copy(ott, ps)
                        nc.sync.dma_start(
                            out=out_d[tb * 128:(tb + 1) * 128,
                                      eb * 128:(eb + 1) * 128], in_=ott)

    nc.compile()
    return nc
# ---------------------------------------------------------------------------
# host side
# ---------------------------------------------------------------------------

_CACHE = {}


def _host_prep(inputs, T=T_FULL):
    ii = {k: np.asarray(v, dtype=np.float32) for k, v in inputs.items()}
    g1, be1, g2, be2, g3, be3 = (ii[k] for k in ("g1", "be1", "g2", "be2", "g3", "be3"))

    def fold_qkv(wstk, g, be, scale=1.0):
        wall = np.transpose(wstk, (1, 0, 2)).reshape(E, H * D)  # [E, H*D]
        return ((g[:, None] * wall) * (scale * 16.0)).astype(ml_dtypes.float8_e4m3), \
               ((be @ wall) * scale).astype(np.float32)

    sc = float(D) ** -0.5
    wq_s, bq_s = fold_qkv(ii["Wq_s"], g1, be1, sc)
    wk_s, bk_s = fold_qkv(ii["Wk_s"], g1, be1)
    wv_s, bv_s = fold_qkv(ii["Wv_s"], g1, be1)
    wq_c, bq_c = fold_qkv(ii["Wq_c"], g2, be2, sc)
    wk_c, bk_c = fold_qkv(ii["Wk_c"], np.ones(E, np.float32), np.zeros(E, np.float32))
    wv_c, bv_c = fold_qkv(ii["Wv_c"], np.ones(E, np.float32), np.zeros(E, np.float32))
    assert np.allclose(bv_s, 0, atol=1e-6) and np.allclose(bv_c, 0, atol=1e-6), \
        "V-projection bias folding not implemented (be nonzero)"
    w1 = (g3[:, None] * ii["W1"] * 16.0).astype(ml_dtypes.float8_e4m3)
    b1f = ((be3 @ ii["W1"] + ii["b1"]) * 16.0).astype(np.float32)
    w2 = (ii["W2"] * 16.0).astype(ml_dtypes.float8_e4m3)

    shared = dict(
        wq_s=wq_s, wk_s=wk_s, wv_s=wv_s, wp_s=ii["Wp_s"].astype(ml_dtypes.bfloat16),
        wq_c=wq_c, wk_c=wk_c, wv_c=wv_c, wp_c=ii["Wp_c"].astype(ml_dtypes.bfloat16),
        w1=w1, w2=w2, b1f=b1f,
        bq_s=bq_s, bk_s=bk_s, bq_c=bq_c, bk_c=bk_c,
        bp_s=ii["bp_s"].astype(np.float32), bp_c=ii["bp_c"].astype(np.float32),
        ident=np.eye(128, dtype=np.float32),
    )
    mtri = np.triu(np.ones((128, 128), dtype=np.float32))

    q, k, v = ii["q"], ii["k"], ii["v"]
    n_b = q.shape[0]
    n_stripes = T // 128
    in_maps = []
    for core in range(2 * n_b):
        b, p = core // 2, core % 2
        order = [2 * i + (1 - p) for i in range(n_stripes // 2)] + \
                [2 * i + p for i in range(n_stripes // 2)]
        cols = np.concatenate([np.arange(s * 128, (s + 1) * 128) for s in order])
        m = dict(shared)
        m["qTp"] = np.ascontiguousarray(q[b].T[:, cols])
        m["kT"] = np.ascontiguousarray(k[b].T).astype(ml_dtypes.float8_e4m3)
        m["vT"] = np.ascontiguousarray(v[b].T).astype(ml_dtypes.float8_e4m3)
        m["msk2"] = np.hstack([np.full((128, 128), float(p), np.float32),
                               mtri]).astype(ml_dtypes.bfloat16)
        in_maps.append(m)
    return in_maps


def _gather(results, b2, T=T_FULL, n_b=B):
    out = np.zeros((n_b, T, E), dtype=np.float32)
    for core in range(2 * n_b):
        b, p = core // 2, core % 2
        r = results[core]["out"].reshape(T // 256, 128, E)
        for j in range(T // 256):
            out[b, (2 * j + p) * 128:(2 * j + p + 1) * 128, :] = r[j]
    return out + np.asarray(b2, np.float32)   # b2 bias folded on host


def kernel(**inputs):
    if "nc" not in _CACHE:
        _CACHE["nc"] = build_nc(T_FULL)
    nc = _CACHE["nc"]
    in_maps = _host_prep(inputs, T_FULL)
    res = run_bass_kernel_spmd(nc, in_maps, core_ids=list(range(NCORES)))
    return _gather(res.results, inputs["b2"], T_FULL)



# revision 45
# speedup vs baseline: 1.0606x; 1.0606x over previous
"""Trainium2 Bass kernel for a dense transformer DecoderLayer.

Layer: x = q
  x += SelfAttn(LN1(x))   (causal, 8 heads)
  x += CrossAttn(LN2(x), k, v)
  x += FFN(LN3(x))        (E -> 4E relu -> E)

Sharding: 8 cores = (batch b = core//2, parity p = core%2). Core (b, p)
owns the 8 odd-or-even 128-row stripes of batch b's 2048 query rows.
Host permutes q[b]^T columns to [partner stripes | own stripes] so the
device program is identical on every core (SPMD); the parity-dependent
causal boundary is carried by a data mask (mp = all-ones or all-zeros).

Device dataflow is fully "transposed": the residual stream lives as
x^T [E=512 partitions(4 tiles), tokens] so no on-device activation
transposes are needed except the final 128x128 PE transposes on output.
Scores are computed transposed (S^T [s,q]) so the softmax denominator
falls out of the P@V matmul via an appended ones-column on V.
"""

import numpy as np
import ml_dtypes

import concourse.bass as bass
import concourse.tile as tile
from concourse import bacc
from concourse import mybir
from concourse.bass_utils import run_bass_kernel_spmd

F32 = mybir.dt.float32
F32R = mybir.dt.float32r
BF16 = mybir.dt.bfloat16
FP8 = mybir.dt.float8e4

B, T_FULL, E, H, D, FW = 4, 2048, 512, 8, 64, 4
EC = E // 128           # e-chunks
F = FW * E              # ffn hidden
FC = F // 128
EPS = 1e-5
NCORES = 8


def _pieces(a, b, step=512):
    """Split [a, b) at multiples of `step` (PSUM-bank aligned pieces)."""
    out = []
    while a < b:
        nxt = min(b, (a // step + 1) * step)
        out.append((a, nxt))
        a = nxt
    return out


def _pieces_bf(a, b):
    """Matmul output pieces: one PSUM bank (512 f32) per matmul — walrus
    rejects bank-spanning matmul outputs."""
    return _pieces(a, b, 512)


def build_nc(T=T_FULL):
    R = T // 2           # own query columns (packed at [R:T])
    G = R // 128         # own 128-col groups
    NCH = T // 128       # total s-chunks

    nc = bacc.Bacc(None, target_bir_lowering=False)

    # ---------------- DRAM I/O ----------------
    qTp = nc.dram_tensor("qTp", [E, T], F32, kind="ExternalInput")
    kT = nc.dram_tensor("kT", [E, T], mybir.dt.float8e4, kind="ExternalInput")
    vT = nc.dram_tensor("vT", [E, T], mybir.dt.float8e4, kind="ExternalInput")
    w = {}
    for nm in ("wq_s", "wk_s", "wv_s", "wq_c", "wk_c", "wv_c"):
        w[nm] = nc.dram_tensor(nm, [E, H * D], mybir.dt.float8e4, kind="ExternalInput")
    w["wp_s"] = nc.dram_tensor("wp_s", [H * D, E], BF16, kind="ExternalInput")
    w["wp_c"] = nc.dram_tensor("wp_c", [H * D, E], BF16, kind="ExternalInput")
    w["w1"] = nc.dram_tensor("w1", [E, F], mybir.dt.float8e4, kind="ExternalInput")
    w["w2"] = nc.dram_tensor("w2", [F, E], mybir.dt.float8e4, kind="ExternalInput")
    bias_d = {}
    for nm, sz in (("bq_s", H * D), ("bk_s", H * D), ("bq_c", H * D), ("bk_c", H * D),
                   ("bp_s", E), ("bp_c", E), ("b1f", F)):
        bias_d[nm] = nc.dram_tensor(nm, [sz], F32, kind="ExternalInput")
    msk2_d = nc.dram_tensor("msk2", [128, 256], BF16, kind="ExternalInput")
    ident_d = nc.dram_tensor("ident", [128, 128], F32, kind="ExternalInput")
    out_d = nc.dram_tensor("out", [R, E], F32, kind="ExternalOutput")

    with tile.TileContext(nc) as tc:
        with (
            tc.tile_pool(name="resident", bufs=1) as res,
            tc.tile_pool(name="consts", bufs=1) as cpool,
            tc.tile_pool(name="work", bufs=2) as work,
            tc.tile_pool(name="es_pool", bufs=4) as es_pool,
            tc.tile_pool(name="stat", bufs=2) as stat,
            tc.tile_pool(name="drampool", bufs=2, space="DRAM") as drampool,
            tc.tile_pool(name="psum", bufs=1, space="PSUM") as psum,
        ):
            # ---- PSUM tags: "sc" 2bk x2, "o" 1bk x2, "st" 1bk x2 = 8 banks
            def ps_sc(name):
                """[128, 2, 512] scores-pair tile; projections use [:, 0, :]."""
                return psum.tile([128, 2, 512], F32, name=name, tag="sc", bufs=2)

            def ps_o(name, shape=None):
                return psum.tile(shape or [128, 512], F32, name=name, tag="o", bufs=2)

            # ---------------- resident loads ----------------
            xto = []        # own-half residual stream [E, R], lives whole kernel
            for c in range(EC):
                t_ = res.tile([128, R], F32, name=f"xto{c}")
                nc.sync.dma_start(
                    out=t_, in_=qTp.rearrange("(c p) t -> c p t", p=128)[c][:, R:T])
                xto.append(t_)

            bias_sb = {}
            for nm in bias_d:
                src = bias_d[nm]
                t_ = cpool.tile([128, src.shape[0] // 128], F32, name=f"b_{nm}")
                nc.sync.dma_start(out=t_, in_=src.rearrange("(c p) -> p c", p=128))
                bias_sb[nm] = t_
            msk2_sb = cpool.tile([128, 2, 128], BF16, name="msk2_sb")
            nc.sync.dma_start(out=msk2_sb, in_=msk2_d[:, :])
            ident_sb = cpool.tile([128, 128], F32, name="ident_sb")
            nc.sync.dma_start(out=ident_sb, in_=ident_d[:, :])
            # fp8 ones/E for DoubleRow LN-stats lhsT (middle stride 16: ok)
            ones8 = cpool.tile([128, 2, 16], FP8, name="ones8")
            nc.vector.memset(ones8, 1.0 / E)
            eps_sb = cpool.tile([1, 1], F32, name="eps_sb")
            nc.vector.memset(eps_sb, EPS)

            def load_w(pool, nm, tag):
                src = w[nm]
                if nm.startswith("wp"):
                    # head-pair packed: rows [hp*128:(hp+1)*128] = heads 2hp,2hp+1
                    t_ = pool.tile([128, H // 2, E], BF16, name=f"sb_{nm}", tag=tag)
                    nc.sync.dma_start(out=t_, in_=src.rearrange("(hp p) n -> p hp n", p=128))
                else:
                    t_ = pool.tile([128, src.shape[0] // 128, src.shape[1]],
                                   src.dtype, name=f"sb_{nm}", tag=tag)
                    nc.sync.dma_start(out=t_, in_=src.rearrange("(c p) n -> p c n", p=128))
                return t_

            # ---------------- transposed layernorm (per 512-token piece) ----
            def ln_half(xap, xn8, tag, qa, qb, prof):
                """LN over token cols [qa, qb) (<=512 wide). xap(c, a, b) ->
                [128, b-a] f32 SBUF AP; writes normalized fp8 to
                xn8[:, c, qa:qb]. prof picks engines: "pre" = Act square
                (idle prefix), "mid" = Pool (Act busy with exp)."""
                wd = qb - qa
                xs = work.tile([128, EC, 512], FP8, name=f"xs_{tag}", tag="lnxs", bufs=2)
                sqt = work.tile([128, EC, 512], FP8, name=f"sq_{tag}", tag="lnsq", bufs=2)
                for c in range(EC):
                    xa = xap(c, qa, qb)
                    if prof == "pre":
                        nc.scalar.activation(sqt[:, c, :wd], xa,
                                             mybir.ActivationFunctionType.Square)
                    else:
                        nc.gpsimd.tensor_mul(sqt[:, c, :wd], xa, xa)
                    nc.gpsimd.tensor_copy(xs[:, c, :wd], xa)
                st0 = psum.tile([1, 512], F32, name=f"st0_{tag}", tag="st", bufs=2)
                st1 = psum.tile([1, 512], F32, name=f"st1_{tag}", tag="st", bufs=2)
                for c in range(0, EC, 2):
                    nc.tensor.matmul(st0[:, :wd], ones8[:, :, 0:1], xs[:, c:c + 2, :wd],
                                     start=(c == 0), stop=(c == EC - 2),
                                     perf_mode=mybir.MatmulPerfMode.DoubleRow,
                                     skip_group_check=True)
                    nc.tensor.matmul(st1[:, :wd], ones8[:, :, 0:1], sqt[:, c:c + 2, :wd],
                                     start=(c == 0), stop=(c == EC - 2),
                                     perf_mode=mybir.MatmulPerfMode.DoubleRow,
                                     skip_group_check=True)
                var = stat.tile([1, 512], F32, name=f"var_{tag}", tag="var", bufs=2)
                m_sb = stat.tile([1, 512], F32, name=f"m_{tag}", tag="m_sb", bufs=2)
                nc.vector.tensor_copy(m_sb[:, :wd], st0[:, :wd])
                nc.vector.tensor_mul(var[:, :wd], m_sb[:, :wd], st0[:, :wd])
                nc.vector.tensor_sub(var[:, :wd], st1[:, :wd], var[:, :wd])
                # rsqrt = exp(-0.5*ln(var+eps)): stays in the ln/exp act table
                nc.scalar.activation(var[:, :wd], var[:, :wd],
                                     mybir.ActivationFunctionType.Ln,
                                     bias=eps_sb[0:1, 0:1])
                nc.scalar.activation(var[:, :wd], var[:, :wd],
                                     mybir.ActivationFunctionType.Exp, scale=-0.5)
                mb = work.tile([128, 512], F32, name=f"mb_{tag}", tag="mb", bufs=2)
                rb = work.tile([128, 512], F32, name=f"rb_{tag}", tag="rb", bufs=2)
                m_dr = drampool.tile([1, 512], F32, name=f"mdr_{tag}", tag="mdr", bufs=3)
                r_dr = drampool.tile([1, 512], F32, name=f"rdr_{tag}", tag="rdr", bufs=3)
                nc.sync.dma_start(out=m_dr[:, :wd], in_=m_sb[:, :wd])
                nc.sync.dma_start(out=r_dr[:, :wd], in_=var[:, :wd])
                nc.sync.dma_start(out=mb[:, :wd], in_=m_dr[:, :wd].to_broadcast((128, wd)))
                nc.sync.dma_start(out=rb[:, :wd], in_=r_dr[:, :wd].to_broadcast((128, wd)))
                for c in range(EC):
                    tmp = work.tile([128, 512], F32, name=f"lt_{tag}", tag="lntmp", bufs=2)
                    nc.vector.tensor_sub(tmp[:, :wd], xap(c, qa, qb), mb[:, :wd])
                    nc.vector.tensor_mul(xn8[:, c, qa:qb], tmp[:, :wd], rb[:, :wd])

            # ---------------- attention building blocks ----------------
            def evac_proj(out_ap, ps_ap, b_ap, eng):
                """PSUM -> SBUF evac: out = ps/16 + bias, on chosen engine."""
                if eng == "act":
                    nc.scalar.activation(out_ap, ps_ap,
                                         mybir.ActivationFunctionType.Identity,
                                         bias=b_ap, scale=1.0 / 16.0)
                else:
                    nc.vector.tensor_scalar(out_ap, ps_ap, 1.0 / 16.0, b_ap,
                                            op0=mybir.AluOpType.mult,
                                            op1=mybir.AluOpType.add)

            def proj_kt(apool, tags, src_aps, wk_t, bk_sb, tag, order=None):
                """K^T head-pair tiles [128, T]; emits all pieces (prefix)."""
                ktp = [apool.tile([128, T], BF16, name=f"ktp_{tag}{pp}", tag=tags[pp])
                       for pp in range(4)]
                pcs = order or _pieces_bf(0, T)
                for (a, b_) in pcs:
                    for pp in range(4):
                        ps = ps_sc(f"kps_{tag}")
                        for ci, c in enumerate(range(0, EC, 2)):
                            nc.tensor.matmul(
                                ps[:, 0, :b_ - a],
                                wk_t[:, c:c + 2, pp * 128:(pp + 1) * 128],
                                src_aps[ci](a, b_),
                                start=(c == 0), stop=(c == EC - 2),
                                perf_mode=mybir.MatmulPerfMode.DoubleRow)
                        evac_proj(ktp[pp][:, a:b_], ps[:, 0, :b_ - a],
                                  bk_sb[:, pp:pp + 1], "act" if pp % 2 == 0 else "dve")
                return ktp

            def proj_qt_half(qtp, xn_q, wq_t, bq_sb, tag, qa, qb, eng, pp=None):
                """Q^T projection for token cols [qa, qb). pp=None -> all 4."""
                for p_ in ([pp] if pp is not None else range(4)):
                    ps = ps_sc(f"qps_{tag}")
                    for ci, c in enumerate(range(0, EC, 2)):
                        nc.tensor.matmul(
                            ps[:, 0, :qb - qa],
                            wq_t[:, c:c + 2, p_ * 128:(p_ + 1) * 128],
                            xn_q[ci](qa, qb),
                            start=(c == 0), stop=(c == EC - 2),
                            perf_mode=mybir.MatmulPerfMode.DoubleRow)
                    e_ = ("act" if p_ % 2 == 0 else "dve") if eng == "mix0" else eng
                    evac_proj(qtp[p_][:, qa:qb], ps[:, 0, :qb - qa],
                              bq_sb[:, p_:p_ + 1], e_)

            def heads_half(ktp, qtp, v_pair, causal, tag, o_out, qa, qb,
                           fillers=None, heads=None):
                """One q-half (cols [qa, qb)) of all 8 heads. v_pair(pr, h) ->
                lhsT AP [128, 2, D+2] fp8.  Scores for a chunk pair share one
                [128, 2, 512] PSUM tile -> one exp -> fp8 es -> DoubleRow P@V.
                fillers[h] = callables emitted after head h (fill idle engines
                while Act grinds exp)."""
                VP = D + 2
                wd0 = qb - qa
                for h in (heads if heads is not None else range(H)):
                    pp, hr = h // 2, (h % 2) * 64
                    o_ps = ps_o(f"ops_{tag}", [VP, 512])
                    prs = [g for g in range(G) if (not causal) or g * 128 < qb]
                    for pi, pr in enumerate(prs):
                        if causal:
                            kA, kB, q0 = pr, G + pr, pr * 128
                        else:
                            kA, kB, q0 = 2 * pr, 2 * pr + 1, 0
                        a = max(q0, qa)
                        wd = qb - a
                        sc = ps_sc(f"scps_{tag}")
                        nc.tensor.matmul(
                            sc[:, 0, :wd],
                            ktp[pp][hr:hr + 64, kA * 128:(kA + 1) * 128],
                            qtp[pp][hr:hr + 64, a:qb],
                            start=True, stop=True)
                        nc.tensor.matmul(
                            sc[:, 1, :wd],
                            ktp[pp][hr:hr + 64, kB * 128:(kB + 1) * 128],
                            qtp[pp][hr:hr + 64, a:qb],
                            start=True, stop=True)
                        es = es_pool.tile([128, 2, 512], FP8,
                                          name=f"es_{tag}", tag="es")
                        nc.scalar.activation(es[:, :, :wd], sc[:, :, :wd],
                                             mybir.ActivationFunctionType.Exp)
                        if causal and q0 >= qa:
                            # diagonal block: [mp | tri] masks in one Pool op
                            nc.gpsimd.tensor_mul(es[:, :, 0:128],
                                                 es[:, :, 0:128], msk2_sb)
                        nc.tensor.matmul(
                            o_ps[:, a - qa:wd0], v_pair(pr, h), es[:, :, :wd],
                            start=(pi == 0), stop=(pi == len(prs) - 1),
                            perf_mode=mybir.MatmulPerfMode.DoubleRow,
                            skip_group_check=True)
                    dn = stat.tile([1, 512], F32, name=f"dn_{tag}", tag="dn")
                    nc.vector.reciprocal(dn[:, :wd0], o_ps[D:D + 1, :wd0])
                    rb_h = work.tile([64, 512], F32, name=f"rbh_{tag}", tag="rbh", bufs=2)
                    nc.gpsimd.partition_broadcast(rb_h[:, :wd0], dn[:, :wd0])
                    nc.vector.tensor_mul(o_out[pp][hr:hr + 64, qa:qb],
                                         o_ps[0:D, :wd0], rb_h[:, :wd0])
                    if fillers and h in fillers:
                        for f_ in fillers[h]:
                            f_()

            def out_proj_eb(o_pairs, wp_t, bp_sb, tag, qa, qb, eb):
                ps = ps_o(f"yps_{tag}")
                for j in range(H // 2):
                    nc.tensor.matmul(
                        ps[:, :qb - qa],
                        wp_t[:, j, eb * 128:(eb + 1) * 128],
                        o_pairs[j][:, qa:qb],
                        start=(j == 0), stop=(j == H // 2 - 1))
                nc.vector.scalar_tensor_tensor(
                    xto[eb][:, qa:qb], ps[:, :qb - qa], bp_sb[:, eb:eb + 1],
                    xto[eb][:, qa:qb],
                    op0=mybir.AluOpType.add, op1=mybir.AluOpType.add)

            # ================ forward ================
            with tc.tile_pool(name="w_attn", bufs=1) as wat, \
                 tc.tile_pool(name="p_self", bufs=1) as pself:
                # partner-half of residual input (dies after LN1)
                xtp = []
                for c in range(EC):
                    t_ = pself.tile([128, R], F32, name=f"xtp{c}", tag=f"xtp{c}")
                    nc.sync.dma_start(
                        out=t_, in_=qTp.rearrange("(c p) t -> c p t", p=128)[c][:, 0:R])
                    xtp.append(t_)

                wq_t = load_w(wat, "wq_s", "wq")
                wk_t = load_w(wat, "wk_s", "wk")
                wv_t = load_w(wat, "wv_s", "wv")

                def xap1(c, a, b_):
                    if b_ <= R:
                        return xtp[c][:, a:b_]
                    return xto[c][:, a - R:b_ - R]

                # LN1: emit only the pieces heads-h0 needs, defer the rest
                xn1 = pself.tile([128, EC, T], FP8, name="xn1")
                ln_half(xap1, xn1, "ln1", 0, 512, "pre")
                ln_half(xap1, xn1, "ln1", 1024, 1536, "pre")

                # --- self-attn projections: first-need pieces, rest deferred
                ktp_s = [pself.tile([128, T], BF16, name=f"ktp_s{pp}",
                                    tag=f"ktp{pp}") for pp in range(4)]
                xn_k1 = [lambda a, b_, c=c: xn1[:, c:c + 2, a:b_]
                         for c in range(0, EC, 2)]

                def k_step_s(a, b_, pp, eng="dve"):
                    def run():
                        ps = ps_sc("kps_sa")
                        for ci, c in enumerate(range(0, EC, 2)):
                            nc.tensor.matmul(
                                ps[:, 0, :b_ - a],
                                wk_t[:, c:c + 2, pp * 128:(pp + 1) * 128],
                                xn_k1[ci](a, b_),
                                start=(c == 0), stop=(c == EC - 2),
                                perf_mode=mybir.MatmulPerfMode.DoubleRow)
                        evac_proj(ktp_s[pp][:, a:b_], ps[:, 0, :b_ - a],
                                  bias_sb["bk_s"][:, pp:pp + 1], eng)
                    return run

                for (a, b_) in ((0, 512), (1024, 1536)):
                    for pp in range(4):
                        k_step_s(a, b_, pp, "act" if pp % 2 == 0 else "dve")()
                qtp_s = [pself.tile([128, R], BF16, name=f"qtp_s{pp}", tag=f"qtp{pp}")
                         for pp in range(4)]
                xn_q1 = [lambda a, b_, c=c: xn1[:, c:c + 2, R + a:R + b_]
                         for c in range(0, EC, 2)]
                proj_qt_half(qtp_s, xn_q1, wq_t, bias_sb["bq_s"], "sa", 0, 512, "mix0")
                # V chunk-pair tiles [128, 2, H, D+2] fp8: slot 0 = partner
                # stripe g (chunk g), slot 1 = own stripe g (chunk G+g)
                v_sb_s = [pself.tile([128, 2, H, D + 2], FP8, name=f"vpr_sa{g}",
                                     tag=f"vsb{g}") for g in range(G)]

                def v_step_s(k, eng):
                    def run():
                        pair, slot = (k, 0) if k < G else (k - G, 1)
                        ps = ps_sc("vps_sa")
                        for c in range(0, EC, 2):
                            nc.tensor.matmul(
                                ps[:, 0, :512],
                                xn1[:, c:c + 2, k * 128:(k + 1) * 128],
                                wv_t[:, c:c + 2, :],
                                start=(c == 0), stop=(c == EC - 2),
                                perf_mode=mybir.MatmulPerfMode.DoubleRow)
                        vs = v_sb_s[pair]
                        if eng == "act":
                            nc.scalar.activation(
                                vs[:, slot, :, 0:D],
                                ps[:, 0, :512].rearrange("p (h d) -> p h d", h=H),
                                mybir.ActivationFunctionType.Copy, scale=1.0 / 16.0)
                        else:
                            nc.vector.tensor_scalar(
                                vs[:, slot, :, 0:D],
                                ps[:, 0, :512].rearrange("p (h d) -> p h d", h=H),
                                1.0 / 16.0, None, op0=mybir.AluOpType.mult)
                        nc.gpsimd.memset(vs[:, slot, :, D:D + 2], 1.0)
                    return run

                for k in list(range(0, 4)) + list(range(8, 12)):
                    v_step_s(k, "act" if k % 2 == 0 else "dve")()

                # cross/FFN weights: DMA-queued here so they don't delay
                # the LN1 broadcasts, but still arrive well before first use
                wp_t = load_w(wat, "wp_s", "wp")
                wk_ct = load_w(wat, "wk_c", "wk_c")
                wv_ct = load_w(wat, "wv_c", "wv_c")
                wq_ct = load_w(wat, "wq_c", "wq_c")
                wp_ct = load_w(wat, "wp_c", "wp_c")
                w1_t = load_w(wat, "w1", "w1")
                w2_t = load_w(wat, "w2", "w2")

                # --- cross K/V projection steps (fillers) ---
                ktp_c = [pself.tile([128, T], BF16, name=f"ktp_c{pp}",
                                    tag=f"xn1{'abcd'[pp]}") for pp in range(4)]
                kcache = {}

                def ck_step(a, b_, pp):
                    def run():
                        if a not in kcache:
                            t_ = pself.tile([128, EC, 512], FP8, name="ksl_ca",
                                            tag=f"xtp{(a // 512) % 4}", bufs=1)
                            nc.sync.dma_start(
                                out=t_[:, :, :b_ - a],
                                in_=kT.rearrange("(c p) t -> p c t", p=128)[:, :, a:b_])
                            kcache[a] = t_
                        ps = ps_sc("kps_ca")
                        for c in range(0, EC, 2):
                            nc.tensor.matmul(
                                ps[:, 0, :b_ - a],
                                wk_ct[:, c:c + 2, pp * 128:(pp + 1) * 128],
                                kcache[a][:, c:c + 2, :b_ - a],
                                start=(c == 0), stop=(c == EC - 2),
                                perf_mode=mybir.MatmulPerfMode.DoubleRow)
                        evac_proj(ktp_c[pp][:, a:b_], ps[:, 0, :b_ - a],
                                  bias_sb["bk_c"][:, pp:pp + 1], "dve")
                    return run

                v_g = [pself.tile([128, 8, H, D + 2], FP8, name=f"vg{g2}",
                                  tag=f"vg{g2}") for g2 in range(2)]
                vcache = {}

                def cv_step(k):
                    def run():
                        k4 = (k // 4) * 4
                        if k4 not in vcache:
                            vsl = work.tile([128, EC, 512], FP8, name="vsl_ca",
                                            tag="vsl", bufs=2)
                            nc.sync.dma_start(
                                out=vsl,
                                in_=vT.rearrange("(c p) t -> p c t", p=128)
                                [:, :, k4 * 128:(k4 + 4) * 128])
                            vcache[k4] = vsl
                        off = (k - k4) * 128
                        ps = ps_sc("vps_ca")
                        for c in range(0, EC, 2):
                            nc.tensor.matmul(
                                ps[:, 0, :512],
                                vcache[k4][:, c:c + 2, off:off + 128],
                                wv_ct[:, c:c + 2, :],
                                start=(c == 0), stop=(c == EC - 2),
                                perf_mode=mybir.MatmulPerfMode.DoubleRow)
                        nc.vector.tensor_scalar(
                            v_g[k // 8][:, k % 8, :, 0:D],
                            ps[:, 0, :512].rearrange("p (h d) -> p h d", h=H),
                            1.0 / 16.0, None, op0=mybir.AluOpType.mult)
                        nc.gpsimd.memset(v_g[k // 8][:, k % 8, :, D:D + 2], 1.0)
                    return run

                def gen_of(steps):
                    for s_ in steps:
                        s_()
                        yield

                def take(gen, n):
                    def run():
                        for _ in range(n):
                            try:
                                next(gen)
                            except StopIteration:
                                break
                    return run

                # deferred prefix work, ordered so heads-h1 inputs finish first
                rest_steps = [lambda: ln_half(xap1, xn1, "ln1", 512, 1024, "mid")]
                rest_steps += [k_step_s(512, 1024, pp) for pp in range(4)]
                rest_steps.append(lambda: ln_half(xap1, xn1, "ln1", 1536, 2048, "mid"))
                rest_steps += [k_step_s(1536, 2048, pp) for pp in range(4)]
                rest_steps += [lambda pp=pp: proj_qt_half(
                    qtp_s, xn_q1, wq_t, bias_sb["bq_s"], "sa", 512, 1024, "dve", pp)
                    for pp in range(4)]
                rest_steps += [v_step_s(k, "dve")
                               for k in list(range(4, 8)) + list(range(12, 16))]
                rsg = gen_of(rest_steps)
                ckg = gen_of([ck_step(a, b_, pp)
                              for (a, b_) in _pieces_bf(0, T) for pp in range(4)])
                cvg = gen_of([cv_step(k) for k in range(NCH)])

                # --- self attention, q-half pipelined ---
                o1 = [pself.tile([128, R], BF16, name=f"opr1{j}", tag=f"opr{j}")
                      for j in range(H // 2)]
                v_at1 = lambda pr, h: v_sb_s[pr][:, :, h, :]
                fill0 = {0: [take(rsg, 1)], 1: [take(rsg, 4)], 2: [take(rsg, 5)],
                         3: [take(rsg, 4)], 4: [take(rsg, 4)], 5: [take(rsg, 4)],
                         6: [take(rsg, 2), take(ckg, 2)],
                         7: [take(ckg, 3), take(cvg, 3)]}
                heads_half(ktp_s, qtp_s, v_at1, True, "sa", o1, 0, 512, fill0)
                take(rsg, 99)()

                # tail of half0 (outproj + LN2 + Qc) woven into half1
                xn2 = pself.tile([128, EC, R], FP8, name="xn2")
                qtp_ct = [pself.tile([128, R], BF16, name=f"qtp_c{pp}",
                                     tag=f"qtpc{pp}") for pp in range(4)]
                xap2 = lambda c, a, b_: xto[c][:, a:b_]
                xn_q2 = [lambda a, b_, c=c: xn2[:, c:c + 2, a:b_]
                         for c in range(0, EC, 2)]
                t0_steps = [lambda eb=eb: out_proj_eb(o1, wp_t, bias_sb["bp_s"],
                                                      "sa", 0, 512, eb)
                            for eb in range(EC)]
                t0_steps.append(lambda: ln_half(xap2, xn2, "ln2a", 0, 512, "mid"))
                t0_steps += [lambda pp=pp: proj_qt_half(
                    qtp_ct, xn_q2, wq_ct, bias_sb["bq_c"], "ca", 0, 512, "dve", pp)
                    for pp in range(4)]
                t0g = gen_of(t0_steps)
                # o2 aliases qtp_s tags: qtp_s[j] is last read by self-h1 head
                # 2j+1, well before cross-h0 head 2j writes o2[j]
                o2 = [pself.tile([128, R], BF16, name=f"opr2{j}", tag=f"qtp{j}")
                      for j in range(H // 2)]
                v_at2 = lambda pr, h: v_g[(2 * pr) // 8][:, (2 * pr) % 8:(2 * pr) % 8 + 2, h, :]

                def xh0(heads):
                    return lambda: heads_half(ktp_c, qtp_ct, v_at2, False, "ca",
                                              o2, 0, 512, None, heads=heads)

                fill1 = {0: [take(ckg, 6), take(cvg, 3)],
                         1: [take(ckg, 5), take(cvg, 3)],
                         2: [take(cvg, 3), take(t0g, 2)],
                         3: [take(cvg, 4), take(t0g, 2)],
                         4: [take(t0g, 1)], 5: [take(t0g, 2)],
                         6: [take(t0g, 2), xh0([0, 1])],
                         7: [xh0([2, 3])]}
                heads_half(ktp_s, qtp_s, v_at1, True, "sa", o1, 512, 1024, fill1)
                take(ckg, 99)(); take(cvg, 99)(); take(t0g, 99)()

                # half1 tail: out-proj, then rest of cross-h0 with LN2-h1/Qc-h1
                for eb in range(EC):
                    out_proj_eb(o1, wp_t, bias_sb["bp_s"], "sa", 512, 1024, eb)
                tb_steps = [lambda: ln_half(xap2, xn2, "ln2b", 512, 1024, "mid")]
                tb_steps += [lambda pp=pp: proj_qt_half(
                    qtp_ct, xn_q2, wq_ct, bias_sb["bq_c"], "ca", 512, 1024, "dve", pp)
                    for pp in range(4)]
                tbg = gen_of(tb_steps)
                fillc0 = {4: [take(tbg, 1)], 5: [take(tbg, 2)], 6: [take(tbg, 2)]}
                heads_half(ktp_c, qtp_ct, v_at2, False, "ca", o2, 0, 512, fillc0,
                           heads=[4, 5, 6, 7])
                take(tbg, 99)()

                # tail of cross-h0 (outproj + LN3 + FFN-h0) woven into cross-h1
                # tag-alias onto tiles whose last readers are already done:
                # xn3 reuses xn2's space (dead after qtp_c), h1t reuses xn1's
                xn3 = pself.tile([128, EC, R], FP8, name="xn3", tag="xn2")
                h1t = pself.tile([128, FC, R], FP8, name="h1t", tag="xn1")

                def ffn1_f(qa, qb, f, eng):
                    # h0 fillers use the stats bank; the h1 tail can take the
                    # scores tag (attention is finished there)
                    if qa == 0:
                        ps = psum.tile([128, 512], F32, name="hps", tag="st", bufs=2)
                    else:
                        ps = ps_sc("hps")[:, 0, :]
                    for c in range(0, EC, 2):
                        nc.tensor.matmul(
                            ps[:, :qb - qa],
                            w1_t[:, c:c + 2, f * 128:(f + 1) * 128],
                            xn3[:, c:c + 2, qa:qb],
                            start=(c == 0), stop=(c == EC - 2),
                            perf_mode=mybir.MatmulPerfMode.DoubleRow)
                    if eng == "act":
                        nc.scalar.activation(
                            h1t[:, f, qa:qb], ps[:, :qb - qa],
                            mybir.ActivationFunctionType.Relu,
                            bias=bias_sb["b1f"][:, f:f + 1])
                    else:
                        nc.vector.tensor_scalar(
                            h1t[:, f, qa:qb], ps[:, :qb - qa],
                            bias_sb["b1f"][:, f:f + 1], 0.0,
                            op0=mybir.AluOpType.add, op1=mybir.AluOpType.max)

                def ffn2_eb(qa, qb, eb):
                    ps = ps_o("y2ps")
                    for f in range(0, FC, 2):
                        nc.tensor.matmul(
                            ps[:, :qb - qa],
                            w2_t[:, f:f + 2, eb * 128:(eb + 1) * 128],
                            h1t[:, f:f + 2, qa:qb],
                            start=(f == 0), stop=(f == FC - 2),
                            perf_mode=mybir.MatmulPerfMode.DoubleRow)
                    nc.vector.scalar_tensor_tensor(
                        xto[eb][:, qa:qb], ps[:, :qb - qa], 1.0 / 256.0,
                        xto[eb][:, qa:qb],
                        op0=mybir.AluOpType.mult, op1=mybir.AluOpType.add)

                tc0_steps = [lambda eb=eb: out_proj_eb(o2, wp_ct, bias_sb["bp_c"],
                                                       "ca", 0, 512, eb)
                             for eb in range(EC)]
                tc0_steps.append(lambda: ln_half(xap2, xn3, "ln3a", 0, 512, "mid"))
                tc0_steps += [lambda f=f: ffn1_f(0, 512, f, "dve")
                              for f in range(FC)]
                tc0_steps += [lambda eb=eb: ffn2_eb(0, 512, eb) for eb in range(EC)]
                tcg = gen_of(tc0_steps)
                fillc = {0: [take(tcg, 2)], 1: [take(tcg, 3)], 2: [take(tcg, 3)],
                         3: [take(tcg, 3)], 4: [take(tcg, 3)], 5: [take(tcg, 3)],
                         6: [take(tcg, 4)], 7: [take(tcg, 4)]}
                heads_half(ktp_c, qtp_ct, v_at2, False, "ca", o2, 512, 1024, fillc)
                take(tcg, 99)()

                # cross half1 tail + FFN half1 + transpose/store per eb
                for eb in range(EC):
                    out_proj_eb(o2, wp_ct, bias_sb["bp_c"], "ca", 512, 1024, eb)
                ln_half(xap2, xn3, "ln3b", 512, 1024, "pre")
                for f in range(FC):
                    ffn1_f(512, 1024, f, "act" if f % 2 == 0 else "dve")
                for eb in range(EC):
                    ffn2_eb(512, 1024, eb)
                    for tb in range(R // 128):
                        ps = psum.tile([128, 128], F32, name="trp", tag="o", bufs=2)
                        nc.tensor.transpose(
                            ps, xto[eb][:, tb * 128:(tb + 1) * 128], ident_sb)
                        ott = work.tile([128, 128], F32, name="ott", tag="ott", bufs=4)
                        if tb % 2 == 0:
                            nc.vector.tensor_copy(ott, ps)
                        else:
                            nc.scalar.copy(ott, ps)
                        nc.sync.dma_start(
                            out=out_d[tb * 128:(tb + 1) * 128,
                                      eb * 128:(eb + 1) * 128], in_=ott)

    nc.compile()
    return nc
# ---------------------------------------------------------------------------
# host side
# ---------------------------------------------------------------------------

_CACHE = {}


def _host_prep(inputs, T=T_FULL):
    ii = {k: np.asarray(v, dtype=np.float32) for k, v in inputs.items()}
    g1, be1, g2, be2, g3, be3 = (ii[k] for k in ("g1", "be1", "g2", "be2", "g3", "be3"))

    def fold_qkv(wstk, g, be, scale=1.0):
        wall = np.transpose(wstk, (1, 0, 2)).reshape(E, H * D)  # [E, H*D]
        return ((g[:, None] * wall) * (scale * 16.0)).astype(ml_dtypes.float8_e4m3), \
               ((be @ wall) * scale).astype(np.float32)

    sc = float(D) ** -0.5
    wq_s, bq_s = fold_qkv(ii["Wq_s"], g1, be1, sc)
    wk_s, bk_s = fold_qkv(ii["Wk_s"], g1, be1)
    wv_s, bv_s = fold_qkv(ii["Wv_s"], g1, be1)
    wq_c, bq_c = fold_qkv(ii["Wq_c"], g2, be2, sc)
    wk_c, bk_c = fold_qkv(ii["Wk_c"], np.ones(E, np.float32), np.zeros(E, np.float32))
    wv_c, bv_c = fold_qkv(ii["Wv_c"], np.ones(E, np.float32), np.zeros(E, np.float32))
    assert np.allclose(bv_s, 0, atol=1e-6) and np.allclose(bv_c, 0, atol=1e-6), \
        "V-projection bias folding not implemented (be nonzero)"
    w1 = (g3[:, None] * ii["W1"] * 16.0).astype(ml_dtypes.float8_e4m3)
    b1f = ((be3 @ ii["W1"] + ii["b1"]) * 16.0).astype(np.float32)
    w2 = (ii["W2"] * 16.0).astype(ml_dtypes.float8_e4m3)

    shared = dict(
        wq_s=wq_s, wk_s=wk_s, wv_s=wv_s, wp_s=ii["Wp_s"].astype(ml_dtypes.bfloat16),
        wq_c=wq_c, wk_c=wk_c, wv_c=wv_c, wp_c=ii["Wp_c"].astype(ml_dtypes.bfloat16),
        w1=w1, w2=w2, b1f=b1f,
        bq_s=bq_s, bk_s=bk_s, bq_c=bq_c, bk_c=bk_c,
        bp_s=ii["bp_s"].astype(np.float32), bp_c=ii["bp_c"].astype(np.float32),
        ident=np.eye(128, dtype=np.float32),
    )
    mtri = np.triu(np.ones((128, 128), dtype=np.float32))

    q, k, v = ii["q"], ii["k"], ii["v"]
    n_b = q.shape[0]
    n_stripes = T // 128
    in_maps = []
    for core in range(2 * n_b):
        b, p = core // 2, core % 2
        order = [2 * i + (1 - p) for i in range(n_stripes // 2)] + \
                [2 * i + p for i in range(n_stripes // 2)]
        cols = np.concatenate([np.arange(s * 128, (s + 1) * 128) for s in order])
        m = dict(shared)
        m["qTp"] = np.ascontiguousarray(q[b].T[:, cols])
        m["kT"] = np.ascontiguousarray(k[b].T).astype(ml_dtypes.float8_e4m3)
        m["vT"] = np.ascontiguousarray(v[b].T).astype(ml_dtypes.float8_e4m3)
        m["msk2"] = np.hstack([np.full((128, 128), float(p), np.float32),
                               mtri]).astype(ml_dtypes.bfloat16)
        in_maps.append(m)
    return in_maps


def _gather(results, b2, T=T_FULL, n_b=B):
    out = np.zeros((n_b, T, E), dtype=np.float32)
    for core in range(2 * n_b):
        b, p = core // 2, core % 2
        r = results[core]["out"].reshape(T // 256, 128, E)
        for j in range(T // 256):
            out[b, (2 * j + p) * 128:(2 * j + p + 1) * 128, :] = r[j]
    return out + np.asarray(b2, np.float32)   # b2 bias folded on host


def kernel(**inputs):
    if "nc" not in _CACHE:
        _CACHE["nc"] = build_nc(T_FULL)
    nc = _CACHE["nc"]
    in_maps = _host_prep(inputs, T_FULL)
    res = run_bass_kernel_spmd(nc, in_maps, core_ids=list(range(NCORES)))
    return _gather(res.results, inputs["b2"], T_FULL)



# revision 46
# speedup vs baseline: 1.0993x; 1.0365x over previous
"""Trainium2 Bass kernel for a dense transformer DecoderLayer.

Layer: x = q
  x += SelfAttn(LN1(x))   (causal, 8 heads)
  x += CrossAttn(LN2(x), k, v)
  x += FFN(LN3(x))        (E -> 4E relu -> E)

Sharding: 8 cores = (batch b = core//2, parity p = core%2). Core (b, p)
owns the 8 odd-or-even 128-row stripes of batch b's 2048 query rows.
Host permutes q[b]^T columns to [partner stripes | own stripes] so the
device program is identical on every core (SPMD); the parity-dependent
causal boundary is carried by a data mask (mp = all-ones or all-zeros).

Device dataflow is fully "transposed": the residual stream lives as
x^T [E=512 partitions(4 tiles), tokens] so no on-device activation
transposes are needed except the final 128x128 PE transposes on output.
Scores are computed transposed (S^T [s,q]) so the softmax denominator
falls out of the P@V matmul via an appended ones-column on V.
"""

import numpy as np
import ml_dtypes

import concourse.bass as bass
import concourse.tile as tile
from concourse import bacc
from concourse import mybir
from concourse.bass_utils import run_bass_kernel_spmd

F32 = mybir.dt.float32
F32R = mybir.dt.float32r
BF16 = mybir.dt.bfloat16
FP8 = mybir.dt.float8e4

B, T_FULL, E, H, D, FW = 4, 2048, 512, 8, 64, 4
EC = E // 128           # e-chunks
F = FW * E              # ffn hidden
FC = F // 128
EPS = 1e-5
NCORES = 8


def _pieces(a, b, step=512):
    """Split [a, b) at multiples of `step` (PSUM-bank aligned pieces)."""
    out = []
    while a < b:
        nxt = min(b, (a // step + 1) * step)
        out.append((a, nxt))
        a = nxt
    return out


def _pieces_bf(a, b):
    """Matmul output pieces: one PSUM bank (512 f32) per matmul — walrus
    rejects bank-spanning matmul outputs."""
    return _pieces(a, b, 512)


def build_nc(T=T_FULL):
    R = T // 2           # own query columns (packed at [R:T])
    G = R // 128         # own 128-col groups
    NCH = T // 128       # total s-chunks

    nc = bacc.Bacc(None, target_bir_lowering=False)

    # ---------------- DRAM I/O ----------------
    qTp = nc.dram_tensor("qTp", [E, T], F32, kind="ExternalInput")
    kT = nc.dram_tensor("kT", [E, T], mybir.dt.float8e4, kind="ExternalInput")
    vT = nc.dram_tensor("vT", [E, T], mybir.dt.float8e4, kind="ExternalInput")
    w = {}
    for nm in ("wq_s", "wk_s", "wv_s", "wq_c", "wk_c", "wv_c"):
        w[nm] = nc.dram_tensor(nm, [E, H * D], mybir.dt.float8e4, kind="ExternalInput")
    w["wp_s"] = nc.dram_tensor("wp_s", [H * D, E], BF16, kind="ExternalInput")
    w["wp_c"] = nc.dram_tensor("wp_c", [H * D, E], BF16, kind="ExternalInput")
    w["w1"] = nc.dram_tensor("w1", [E, F], mybir.dt.float8e4, kind="ExternalInput")
    w["w2"] = nc.dram_tensor("w2", [F, E], mybir.dt.float8e4, kind="ExternalInput")
    bias_d = {}
    for nm, sz in (("bq_s", H * D), ("bk_s", H * D), ("bq_c", H * D), ("bk_c", H * D),
                   ("bp_s", E), ("bp_c", E), ("b1f", F)):
        bias_d[nm] = nc.dram_tensor(nm, [sz], F32, kind="ExternalInput")
    msk2_d = nc.dram_tensor("msk2", [128, 256], BF16, kind="ExternalInput")
    ident_d = nc.dram_tensor("ident", [128, 128], F32, kind="ExternalInput")
    out_d = nc.dram_tensor("out", [R, E], F32, kind="ExternalOutput")

    with tile.TileContext(nc) as tc:
        with (
            tc.tile_pool(name="resident", bufs=1) as res,
            tc.tile_pool(name="consts", bufs=1) as cpool,
            tc.tile_pool(name="work", bufs=2) as work,
            tc.tile_pool(name="es_pool", bufs=4) as es_pool,
            tc.tile_pool(name="stat", bufs=2) as stat,
            tc.tile_pool(name="drampool", bufs=2, space="DRAM") as drampool,
            tc.tile_pool(name="psum", bufs=1, space="PSUM") as psum,
        ):
            # ---- PSUM tags: "sc" 2bk x2, "o" 1bk x2, "st" 1bk x2 = 8 banks
            def ps_sc(name):
                """[128, 2, 512] scores-pair tile; projections use [:, 0, :]."""
                return psum.tile([128, 2, 512], F32, name=name, tag="sc", bufs=2)

            def ps_o(name, shape=None):
                return psum.tile(shape or [128, 512], F32, name=name, tag="o", bufs=2)

            # ---------------- resident loads ----------------
            xto = []        # own-half residual stream [E, R], lives whole kernel
            for c in range(EC):
                t_ = res.tile([128, R], F32, name=f"xto{c}")
                nc.sync.dma_start(
                    out=t_, in_=qTp.rearrange("(c p) t -> c p t", p=128)[c][:, R:T])
                xto.append(t_)

            bias_sb = {}
            for nm in bias_d:
                src = bias_d[nm]
                t_ = cpool.tile([128, src.shape[0] // 128], F32, name=f"b_{nm}")
                nc.sync.dma_start(out=t_, in_=src.rearrange("(c p) -> p c", p=128))
                bias_sb[nm] = t_
            msk2_sb = cpool.tile([128, 2, 128], BF16, name="msk2_sb")
            nc.sync.dma_start(out=msk2_sb, in_=msk2_d[:, :])
            ident_sb = cpool.tile([128, 128], F32, name="ident_sb")
            nc.sync.dma_start(out=ident_sb, in_=ident_d[:, :])
            # fp8 ones/E for DoubleRow LN-stats lhsT (middle stride 16: ok)
            ones8 = cpool.tile([128, 2, 16], FP8, name="ones8")
            nc.vector.memset(ones8, 1.0 / E)
            eps_sb = cpool.tile([1, 1], F32, name="eps_sb")
            nc.vector.memset(eps_sb, EPS)

            def load_w(pool, nm, tag):
                src = w[nm]
                if nm.startswith("wp"):
                    # head-pair packed: rows [hp*128:(hp+1)*128] = heads 2hp,2hp+1
                    t_ = pool.tile([128, H // 2, E], BF16, name=f"sb_{nm}", tag=tag)
                    nc.sync.dma_start(out=t_, in_=src.rearrange("(hp p) n -> p hp n", p=128))
                else:
                    t_ = pool.tile([128, src.shape[0] // 128, src.shape[1]],
                                   src.dtype, name=f"sb_{nm}", tag=tag)
                    nc.sync.dma_start(out=t_, in_=src.rearrange("(c p) n -> p c n", p=128))
                return t_

            # ---------------- transposed layernorm (per 512-token piece) ----
            def ln_half(xap, xn8, tag, qa, qb, prof):
                """LN over token cols [qa, qb) (<=512 wide). xap(c, a, b) ->
                [128, b-a] f32 SBUF AP; writes normalized fp8 to
                xn8[:, c, qa:qb]. prof picks engines: "pre" = Act square
                (idle prefix), "mid" = Pool (Act busy with exp)."""
                wd = qb - qa
                xs = work.tile([128, EC, 512], FP8, name=f"xs_{tag}", tag="lnxs", bufs=2)
                sqt = work.tile([128, EC, 512], FP8, name=f"sq_{tag}", tag="lnsq", bufs=2)
                for c in range(EC):
                    xa = xap(c, qa, qb)
                    if prof == "pre":
                        nc.scalar.activation(sqt[:, c, :wd], xa,
                                             mybir.ActivationFunctionType.Square)
                    else:
                        nc.gpsimd.tensor_mul(sqt[:, c, :wd], xa, xa)
                    nc.gpsimd.tensor_copy(xs[:, c, :wd], xa)
                st0 = psum.tile([1, 512], F32, name=f"st0_{tag}", tag="st", bufs=2)
                st1 = psum.tile([1, 512], F32, name=f"st1_{tag}", tag="st", bufs=2)
                for c in range(0, EC, 2):
                    nc.tensor.matmul(st0[:, :wd], ones8[:, :, 0:1], xs[:, c:c + 2, :wd],
                                     start=(c == 0), stop=(c == EC - 2),
                                     perf_mode=mybir.MatmulPerfMode.DoubleRow,
                                     skip_group_check=True)
                    nc.tensor.matmul(st1[:, :wd], ones8[:, :, 0:1], sqt[:, c:c + 2, :wd],
                                     start=(c == 0), stop=(c == EC - 2),
                                     perf_mode=mybir.MatmulPerfMode.DoubleRow,
                                     skip_group_check=True)
                var = stat.tile([1, 512], F32, name=f"var_{tag}", tag="var", bufs=2)
                m_sb = stat.tile([1, 512], F32, name=f"m_{tag}", tag="m_sb", bufs=2)
                nc.vector.tensor_copy(m_sb[:, :wd], st0[:, :wd])
                nc.vector.tensor_mul(var[:, :wd], m_sb[:, :wd], st0[:, :wd])
                nc.vector.tensor_sub(var[:, :wd], st1[:, :wd], var[:, :wd])
                # rsqrt = exp(-0.5*ln(var+eps)): stays in the ln/exp act table
                nc.scalar.activation(var[:, :wd], var[:, :wd],
                                     mybir.ActivationFunctionType.Ln,
                                     bias=eps_sb[0:1, 0:1])
                nc.scalar.activation(var[:, :wd], var[:, :wd],
                                     mybir.ActivationFunctionType.Exp, scale=-0.5)
                mb = work.tile([128, 512], F32, name=f"mb_{tag}", tag="mb", bufs=2)
                rb = work.tile([128, 512], F32, name=f"rb_{tag}", tag="rb", bufs=2)
                m_dr = drampool.tile([1, 512], F32, name=f"mdr_{tag}", tag="mdr", bufs=3)
                r_dr = drampool.tile([1, 512], F32, name=f"rdr_{tag}", tag="rdr", bufs=3)
                nc.sync.dma_start(out=m_dr[:, :wd], in_=m_sb[:, :wd])
                nc.sync.dma_start(out=r_dr[:, :wd], in_=var[:, :wd])
                nc.sync.dma_start(out=mb[:, :wd], in_=m_dr[:, :wd].to_broadcast((128, wd)))
                nc.sync.dma_start(out=rb[:, :wd], in_=r_dr[:, :wd].to_broadcast((128, wd)))
                for c in range(EC):
                    tmp = work.tile([128, 512], F32, name=f"lt_{tag}", tag="lntmp", bufs=2)
                    nc.vector.tensor_sub(tmp[:, :wd], xap(c, qa, qb), mb[:, :wd])
                    nc.vector.tensor_mul(xn8[:, c, qa:qb], tmp[:, :wd], rb[:, :wd])

            # ---------------- attention building blocks ----------------
            def evac_proj(out_ap, ps_ap, b_ap, eng):
                """PSUM -> SBUF evac: out = ps/16 + bias, on chosen engine."""
                if eng == "act":
                    nc.scalar.activation(out_ap, ps_ap,
                                         mybir.ActivationFunctionType.Identity,
                                         bias=b_ap, scale=1.0 / 16.0)
                else:
                    nc.vector.tensor_scalar(out_ap, ps_ap, 1.0 / 16.0, b_ap,
                                            op0=mybir.AluOpType.mult,
                                            op1=mybir.AluOpType.add)

            def proj_kt(apool, tags, src_aps, wk_t, bk_sb, tag, order=None):
                """K^T head-pair tiles [128, T]; emits all pieces (prefix)."""
                ktp = [apool.tile([128, T], BF16, name=f"ktp_{tag}{pp}", tag=tags[pp])
                       for pp in range(4)]
                pcs = order or _pieces_bf(0, T)
                for (a, b_) in pcs:
                    for pp in range(4):
                        ps = ps_sc(f"kps_{tag}")
                        for ci, c in enumerate(range(0, EC, 2)):
                            nc.tensor.matmul(
                                ps[:, 0, :b_ - a],
                                wk_t[:, c:c + 2, pp * 128:(pp + 1) * 128],
                                src_aps[ci](a, b_),
                                start=(c == 0), stop=(c == EC - 2),
                                perf_mode=mybir.MatmulPerfMode.DoubleRow)
                        evac_proj(ktp[pp][:, a:b_], ps[:, 0, :b_ - a],
                                  bk_sb[:, pp:pp + 1], "act" if pp % 2 == 0 else "dve")
                return ktp

            def proj_qt_half(qtp, xn_q, wq_t, bq_sb, tag, qa, qb, eng, pp=None):
                """Q^T projection for token cols [qa, qb). pp=None -> all 4."""
                for p_ in ([pp] if pp is not None else range(4)):
                    ps = ps_sc(f"qps_{tag}")
                    for ci, c in enumerate(range(0, EC, 2)):
                        nc.tensor.matmul(
                            ps[:, 0, :qb - qa],
                            wq_t[:, c:c + 2, p_ * 128:(p_ + 1) * 128],
                            xn_q[ci](qa, qb),
                            start=(c == 0), stop=(c == EC - 2),
                            perf_mode=mybir.MatmulPerfMode.DoubleRow)
                    e_ = ("act" if p_ % 2 == 0 else "dve") if eng == "mix0" else eng
                    evac_proj(qtp[p_][:, qa:qb], ps[:, 0, :qb - qa],
                              bq_sb[:, p_:p_ + 1], e_)

            def heads_half(ktp, qtp, v_pair, causal, tag, o_out, qa, qb,
                           fillers=None, heads=None):
                """One q-half (cols [qa, qb)) of 8 heads. v_pair(pr, h) ->
                lhsT AP [128, 2, D+2] fp8. Scores for a chunk pair share one
                [128, 2, 512] PSUM tile -> one exp -> fp8 es -> DoubleRow P@V.
                Scores/exp are emitted LAG pairs ahead of P@V so a P@V stalled
                on the o-buffer rotation cannot block upcoming scores (and so
                starve the Act engine). fillers[h]: callables emitted after
                head h's last scores."""
                VP = D + 2
                wd0 = qb - qa
                sc_steps, pv_steps = [], []
                for h in (heads if heads is not None else range(H)):
                    pp, hr = h // 2, (h % 2) * 64
                    prs = [g for g in range(G) if (not causal) or g * 128 < qb]
                    o_cell = {}

                    def mk_sc(h=h, pp=pp, hr=hr, cell=None):
                        def run():
                            pr, es_cell = cell["pr"], cell
                            if causal:
                                kA, kB, q0 = pr, G + pr, pr * 128
                            else:
                                kA, kB, q0 = 2 * pr, 2 * pr + 1, 0
                            a = max(q0, qa)
                            wd = qb - a
                            sc = ps_sc(f"scps_{tag}")
                            nc.tensor.matmul(
                                sc[:, 0, :wd],
                                ktp[pp][hr:hr + 64, kA * 128:(kA + 1) * 128],
                                qtp[pp][hr:hr + 64, a:qb],
                                start=True, stop=True)
                            nc.tensor.matmul(
                                sc[:, 1, :wd],
                                ktp[pp][hr:hr + 64, kB * 128:(kB + 1) * 128],
                                qtp[pp][hr:hr + 64, a:qb],
                                start=True, stop=True)
                            es = es_pool.tile([128, 2, 512], FP8,
                                              name=f"es_{tag}", tag="es")
                            nc.scalar.activation(es[:, :, :wd], sc[:, :, :wd],
                                                 mybir.ActivationFunctionType.Exp)
                            if causal and q0 >= qa:
                                nc.gpsimd.tensor_mul(es[:, :, 0:128],
                                                     es[:, :, 0:128], msk2_sb)
                            es_cell["es"], es_cell["a"], es_cell["wd"] = es, a, wd
                        return run

                    def mk_pv(h=h, pp=pp, hr=hr, o_cell=o_cell, cell=None,
                              first=False, last=False):
                        def run():
                            pr, es, a, wd = cell["pr"], cell["es"], cell["a"], cell["wd"]
                            if first:
                                o_cell["ps"] = ps_o(f"ops_{tag}", [VP, 512])
                            o_ps = o_cell["ps"]
                            nc.tensor.matmul(
                                o_ps[:, a - qa:wd0], v_pair(pr, h), es[:, :, :wd],
                                start=first, stop=last,
                                perf_mode=mybir.MatmulPerfMode.DoubleRow,
                                skip_group_check=True)
                            if last:
                                dn = stat.tile([1, 512], F32, name=f"dn_{tag}", tag="dn")
                                nc.vector.reciprocal(dn[:, :wd0], o_ps[D:D + 1, :wd0])
                                rb_h = work.tile([64, 512], F32, name=f"rbh_{tag}",
                                                 tag="rbh", bufs=2)
                                nc.gpsimd.partition_broadcast(rb_h[:, :wd0], dn[:, :wd0])
                                nc.vector.tensor_mul(o_out[pp][hr:hr + 64, qa:qb],
                                                     o_ps[0:D, :wd0], rb_h[:, :wd0])
                        return run

                    for pi, pr in enumerate(prs):
                        cell = {"pr": pr}
                        sc_steps.append(mk_sc(cell=cell))
                        pv_steps.append(mk_pv(cell=cell, first=(pi == 0),
                                              last=(pi == len(prs) - 1)))
                    sc_steps.append(("fill", h))
                    pv_steps.append(None)

                LAG = 2
                si = pi_ = 0
                emitted_pv = 0
                flat_sc = sc_steps
                # walk: emit sc steps; pv steps trail LAG real-sc-steps behind
                real_idx = []
                for i, s_ in enumerate(flat_sc):
                    if not isinstance(s_, tuple):
                        real_idx.append(i)
                pv_of = {}   # sc position -> pv closure
                k = 0
                for i, s_ in enumerate(flat_sc):
                    if not isinstance(s_, tuple):
                        pv_of[i] = pv_steps[k]
                    k += 1 if True else 0
                # simpler: rebuild aligned lists
                sc_only = [s_ for s_ in flat_sc if not isinstance(s_, tuple)]
                pv_only = [p_ for p_ in pv_steps if p_ is not None]
                fill_pos = {}   # index into sc_only after which filler h runs
                cnt = 0
                for s_ in flat_sc:
                    if isinstance(s_, tuple):
                        fill_pos[cnt - 1] = s_[1]
                    else:
                        cnt += 1
                n = len(sc_only)
                for i in range(n + LAG):
                    if i < n:
                        sc_only[i]()
                    if i - LAG >= 0:
                        pv_only[i - LAG]()
                    if i < n and i in fill_pos and fillers:
                        for f_ in fillers.get(fill_pos[i], []):
                            f_()

            def out_proj_eb(o_pairs, wp_t, bp_sb, tag, qa, qb, eb):
                ps = ps_o(f"yps_{tag}")
                for j in range(H // 2):
                    nc.tensor.matmul(
                        ps[:, :qb - qa],
                        wp_t[:, j, eb * 128:(eb + 1) * 128],
                        o_pairs[j][:, qa:qb],
                        start=(j == 0), stop=(j == H // 2 - 1))
                nc.vector.scalar_tensor_tensor(
                    xto[eb][:, qa:qb], ps[:, :qb - qa], bp_sb[:, eb:eb + 1],
                    xto[eb][:, qa:qb],
                    op0=mybir.AluOpType.add, op1=mybir.AluOpType.add)

            # ================ forward ================
            with tc.tile_pool(name="w_attn", bufs=1) as wat, \
                 tc.tile_pool(name="p_self", bufs=1) as pself:
                # partner-half of residual input (dies after LN1)
                xtp = []
                for c in range(EC):
                    t_ = pself.tile([128, R], F32, name=f"xtp{c}", tag=f"xtp{c}")
                    nc.sync.dma_start(
                        out=t_, in_=qTp.rearrange("(c p) t -> c p t", p=128)[c][:, 0:R])
                    xtp.append(t_)

                wq_t = load_w(wat, "wq_s", "wq")
                wk_t = load_w(wat, "wk_s", "wk")
                wv_t = load_w(wat, "wv_s", "wv")

                def xap1(c, a, b_):
                    if b_ <= R:
                        return xtp[c][:, a:b_]
                    return xto[c][:, a - R:b_ - R]

                # LN1: emit only the pieces heads-h0 needs, defer the rest
                xn1 = pself.tile([128, EC, T], FP8, name="xn1")
                ln_half(xap1, xn1, "ln1", 0, 512, "pre")
                ln_half(xap1, xn1, "ln1", 1024, 1536, "pre")

                # --- self-attn projections: first-need pieces, rest deferred
                ktp_s = [pself.tile([128, T], BF16, name=f"ktp_s{pp}",
                                    tag=f"ktp{pp}") for pp in range(4)]
                xn_k1 = [lambda a, b_, c=c: xn1[:, c:c + 2, a:b_]
                         for c in range(0, EC, 2)]

                def k_step_s(a, b_, pp, eng="dve"):
                    def run():
                        ps = ps_sc("kps_sa")
                        for ci, c in enumerate(range(0, EC, 2)):
                            nc.tensor.matmul(
                                ps[:, 0, :b_ - a],
                                wk_t[:, c:c + 2, pp * 128:(pp + 1) * 128],
                                xn_k1[ci](a, b_),
                                start=(c == 0), stop=(c == EC - 2),
                                perf_mode=mybir.MatmulPerfMode.DoubleRow)
                        evac_proj(ktp_s[pp][:, a:b_], ps[:, 0, :b_ - a],
                                  bias_sb["bk_s"][:, pp:pp + 1], eng)
                    return run

                for (a, b_) in ((0, 512), (1024, 1536)):
                    for pp in range(4):
                        k_step_s(a, b_, pp, "act" if pp % 2 == 0 else "dve")()
                qtp_s = [pself.tile([128, R], BF16, name=f"qtp_s{pp}", tag=f"qtp{pp}")
                         for pp in range(4)]
                xn_q1 = [lambda a, b_, c=c: xn1[:, c:c + 2, R + a:R + b_]
                         for c in range(0, EC, 2)]
                proj_qt_half(qtp_s, xn_q1, wq_t, bias_sb["bq_s"], "sa", 0, 512, "mix0")
                # V chunk-pair tiles [128, 2, H, D+2] fp8: slot 0 = partner
                # stripe g (chunk g), slot 1 = own stripe g (chunk G+g)
                v_sb_s = [pself.tile([128, 2, H, D + 2], FP8, name=f"vpr_sa{g}",
                                     tag=f"vsb{g}") for g in range(G)]

                def v_step_s(k, eng):
                    def run():
                        pair, slot = (k, 0) if k < G else (k - G, 1)
                        ps = ps_sc("vps_sa")
                        for c in range(0, EC, 2):
                            nc.tensor.matmul(
                                ps[:, 0, :512],
                                xn1[:, c:c + 2, k * 128:(k + 1) * 128],
                                wv_t[:, c:c + 2, :],
                                start=(c == 0), stop=(c == EC - 2),
                                perf_mode=mybir.MatmulPerfMode.DoubleRow)
                        vs = v_sb_s[pair]
                        if eng == "act":
                            nc.scalar.activation(
                                vs[:, slot, :, 0:D],
                                ps[:, 0, :512].rearrange("p (h d) -> p h d", h=H),
                                mybir.ActivationFunctionType.Copy, scale=1.0 / 16.0)
                        else:
                            nc.vector.tensor_scalar(
                                vs[:, slot, :, 0:D],
                                ps[:, 0, :512].rearrange("p (h d) -> p h d", h=H),
                                1.0 / 16.0, None, op0=mybir.AluOpType.mult)
                        nc.gpsimd.memset(vs[:, slot, :, D:D + 2], 1.0)
                    return run

                for k in list(range(0, 4)) + list(range(8, 12)):
                    v_step_s(k, "act" if k % 2 == 0 else "dve")()

                # cross/FFN weights: DMA-queued here so they don't delay
                # the LN1 broadcasts, but still arrive well before first use
                wp_t = load_w(wat, "wp_s", "wp")
                wk_ct = load_w(wat, "wk_c", "wk_c")
                wv_ct = load_w(wat, "wv_c", "wv_c")
                wq_ct = load_w(wat, "wq_c", "wq_c")
                wp_ct = load_w(wat, "wp_c", "wp_c")
                w1_t = load_w(wat, "w1", "w1")
                w2_t = load_w(wat, "w2", "w2")

                # --- cross K/V projection steps (fillers) ---
                ktp_c = [pself.tile([128, T], BF16, name=f"ktp_c{pp}",
                                    tag=f"xn1{'abcd'[pp]}") for pp in range(4)]
                kcache = {}

                def ck_step(a, b_, pp):
                    def run():
                        if a not in kcache:
                            t_ = pself.tile([128, EC, 512], FP8, name="ksl_ca",
                                            tag=f"xtp{(a // 512) % 4}", bufs=1)
                            nc.sync.dma_start(
                                out=t_[:, :, :b_ - a],
                                in_=kT.rearrange("(c p) t -> p c t", p=128)[:, :, a:b_])
                            kcache[a] = t_
                        ps = ps_sc("kps_ca")
                        for c in range(0, EC, 2):
                            nc.tensor.matmul(
                                ps[:, 0, :b_ - a],
                                wk_ct[:, c:c + 2, pp * 128:(pp + 1) * 128],
                                kcache[a][:, c:c + 2, :b_ - a],
                                start=(c == 0), stop=(c == EC - 2),
                                perf_mode=mybir.MatmulPerfMode.DoubleRow)
                        evac_proj(ktp_c[pp][:, a:b_], ps[:, 0, :b_ - a],
                                  bias_sb["bk_c"][:, pp:pp + 1], "dve")
                    return run

                v_g = [pself.tile([128, 8, H, D + 2], FP8, name=f"vg{g2}",
                                  tag=f"vg{g2}") for g2 in range(2)]
                vcache = {}

                def cv_step(k):
                    def run():
                        k4 = (k // 4) * 4
                        if k4 not in vcache:
                            vsl = work.tile([128, EC, 512], FP8, name="vsl_ca",
                                            tag="vsl", bufs=2)
                            nc.sync.dma_start(
                                out=vsl,
                                in_=vT.rearrange("(c p) t -> p c t", p=128)
                                [:, :, k4 * 128:(k4 + 4) * 128])
                            vcache[k4] = vsl
                        off = (k - k4) * 128
                        ps = ps_sc("vps_ca")
                        for c in range(0, EC, 2):
                            nc.tensor.matmul(
                                ps[:, 0, :512],
                                vcache[k4][:, c:c + 2, off:off + 128],
                                wv_ct[:, c:c + 2, :],
                                start=(c == 0), stop=(c == EC - 2),
                                perf_mode=mybir.MatmulPerfMode.DoubleRow)
                        nc.vector.tensor_scalar(
                            v_g[k // 8][:, k % 8, :, 0:D],
                            ps[:, 0, :512].rearrange("p (h d) -> p h d", h=H),
                            1.0 / 16.0, None, op0=mybir.AluOpType.mult)
                        nc.gpsimd.memset(v_g[k // 8][:, k % 8, :, D:D + 2], 1.0)
                    return run

                def gen_of(steps):
                    for s_ in steps:
                        s_()
                        yield

                def take(gen, n):
                    def run():
                        for _ in range(n):
                            try:
                                next(gen)
                            except StopIteration:
                                break
                    return run

                # deferred prefix work, ordered so heads-h1 inputs finish first
                rest_steps = [lambda: ln_half(xap1, xn1, "ln1", 512, 1024, "mid")]
                rest_steps += [k_step_s(512, 1024, pp) for pp in range(4)]
                rest_steps.append(lambda: ln_half(xap1, xn1, "ln1", 1536, 2048, "mid"))
                rest_steps += [k_step_s(1536, 2048, pp) for pp in range(4)]
                rest_steps += [lambda pp=pp: proj_qt_half(
                    qtp_s, xn_q1, wq_t, bias_sb["bq_s"], "sa", 512, 1024, "dve", pp)
                    for pp in range(4)]
                rest_steps += [v_step_s(k, "dve")
                               for k in list(range(4, 8)) + list(range(12, 16))]
                rsg = gen_of(rest_steps)
                ckg = gen_of([ck_step(a, b_, pp)
                              for (a, b_) in _pieces_bf(0, T) for pp in range(4)])
                cvg = gen_of([cv_step(k) for k in range(NCH)])

                # --- self attention, q-half pipelined ---
                o1 = [pself.tile([128, R], BF16, name=f"opr1{j}", tag=f"opr{j}")
                      for j in range(H // 2)]
                v_at1 = lambda pr, h: v_sb_s[pr][:, :, h, :]
                fill0 = {0: [take(rsg, 1)], 1: [take(rsg, 4)], 2: [take(rsg, 5)],
                         3: [take(rsg, 4)], 4: [take(rsg, 4)], 5: [take(rsg, 4)],
                         6: [take(rsg, 2), take(ckg, 2)],
                         7: [take(ckg, 3), take(cvg, 3)]}
                heads_half(ktp_s, qtp_s, v_at1, True, "sa", o1, 0, 512, fill0)
                take(rsg, 99)()

                # tail of half0 (outproj + LN2 + Qc) woven into half1
                xn2 = pself.tile([128, EC, R], FP8, name="xn2")
                qtp_ct = [pself.tile([128, R], BF16, name=f"qtp_c{pp}",
                                     tag=f"qtpc{pp}") for pp in range(4)]
                xap2 = lambda c, a, b_: xto[c][:, a:b_]
                xn_q2 = [lambda a, b_, c=c: xn2[:, c:c + 2, a:b_]
                         for c in range(0, EC, 2)]
                t0_steps = [lambda eb=eb: out_proj_eb(o1, wp_t, bias_sb["bp_s"],
                                                      "sa", 0, 512, eb)
                            for eb in range(EC)]
                t0_steps.append(lambda: ln_half(xap2, xn2, "ln2a", 0, 512, "mid"))
                t0_steps += [lambda pp=pp: proj_qt_half(
                    qtp_ct, xn_q2, wq_ct, bias_sb["bq_c"], "ca", 0, 512, "dve", pp)
                    for pp in range(4)]
                t0g = gen_of(t0_steps)
                # o2 aliases qtp_s tags: qtp_s[j] is last read by self-h1 head
                # 2j+1, well before cross-h0 head 2j writes o2[j]
                o2 = [pself.tile([128, R], BF16, name=f"opr2{j}", tag=f"qtp{j}")
                      for j in range(H // 2)]
                v_at2 = lambda pr, h: v_g[(2 * pr) // 8][:, (2 * pr) % 8:(2 * pr) % 8 + 2, h, :]

                def xh0(heads):
                    return lambda: heads_half(ktp_c, qtp_ct, v_at2, False, "ca",
                                              o2, 0, 512, None, heads=heads)

                fill1 = {0: [take(ckg, 6), take(cvg, 3)],
                         1: [take(ckg, 5), take(cvg, 3)],
                         2: [take(cvg, 3), take(t0g, 2)],
                         3: [take(cvg, 4), take(t0g, 2)],
                         4: [take(t0g, 1)], 5: [take(t0g, 2)],
                         6: [take(t0g, 2), xh0([0, 1])],
                         7: [xh0([2, 3])]}
                heads_half(ktp_s, qtp_s, v_at1, True, "sa", o1, 512, 1024, fill1)
                take(ckg, 99)(); take(cvg, 99)(); take(t0g, 99)()

                # half1 tail: out-proj, then rest of cross-h0 with LN2-h1/Qc-h1
                for eb in range(EC):
                    out_proj_eb(o1, wp_t, bias_sb["bp_s"], "sa", 512, 1024, eb)
                tb_steps = [lambda: ln_half(xap2, xn2, "ln2b", 512, 1024, "mid")]
                tb_steps += [lambda pp=pp: proj_qt_half(
                    qtp_ct, xn_q2, wq_ct, bias_sb["bq_c"], "ca", 512, 1024, "dve", pp)
                    for pp in range(4)]
                tbg = gen_of(tb_steps)
                fillc0 = {4: [take(tbg, 1)], 5: [take(tbg, 2)], 6: [take(tbg, 2)]}
                heads_half(ktp_c, qtp_ct, v_at2, False, "ca", o2, 0, 512, fillc0,
                           heads=[4, 5, 6, 7])
                take(tbg, 99)()

                # tail of cross-h0 (outproj + LN3 + FFN-h0) woven into cross-h1
                # tag-alias onto tiles whose last readers are already done:
                # xn3 reuses xn2's space (dead after qtp_c), h1t reuses xn1's
                xn3 = pself.tile([128, EC, R], FP8, name="xn3", tag="xn2")
                h1t = pself.tile([128, FC, R], FP8, name="h1t", tag="xn1")

                def ffn1_f(qa, qb, f, eng):
                    # h0 fillers use the stats bank; the h1 tail can take the
                    # scores tag (attention is finished there)
                    if qa == 0:
                        ps = psum.tile([128, 512], F32, name="hps", tag="st", bufs=2)
                    else:
                        ps = ps_sc("hps")[:, 0, :]
                    for c in range(0, EC, 2):
                        nc.tensor.matmul(
                            ps[:, :qb - qa],
                            w1_t[:, c:c + 2, f * 128:(f + 1) * 128],
                            xn3[:, c:c + 2, qa:qb],
                            start=(c == 0), stop=(c == EC - 2),
                            perf_mode=mybir.MatmulPerfMode.DoubleRow)
                    if eng == "act":
                        nc.scalar.activation(
                            h1t[:, f, qa:qb], ps[:, :qb - qa],
                            mybir.ActivationFunctionType.Relu,
                            bias=bias_sb["b1f"][:, f:f + 1])
                    else:
                        nc.vector.tensor_scalar(
                            h1t[:, f, qa:qb], ps[:, :qb - qa],
                            bias_sb["b1f"][:, f:f + 1], 0.0,
                            op0=mybir.AluOpType.add, op1=mybir.AluOpType.max)

                def ffn2_eb(qa, qb, eb):
                    ps = ps_o("y2ps")
                    for f in range(0, FC, 2):
                        nc.tensor.matmul(
                            ps[:, :qb - qa],
                            w2_t[:, f:f + 2, eb * 128:(eb + 1) * 128],
                            h1t[:, f:f + 2, qa:qb],
                            start=(f == 0), stop=(f == FC - 2),
                            perf_mode=mybir.MatmulPerfMode.DoubleRow)
                    nc.vector.scalar_tensor_tensor(
                        xto[eb][:, qa:qb], ps[:, :qb - qa], 1.0 / 256.0,
                        xto[eb][:, qa:qb],
                        op0=mybir.AluOpType.mult, op1=mybir.AluOpType.add)

                tc0_steps = [lambda eb=eb: out_proj_eb(o2, wp_ct, bias_sb["bp_c"],
                                                       "ca", 0, 512, eb)
                             for eb in range(EC)]
                tc0_steps.append(lambda: ln_half(xap2, xn3, "ln3a", 0, 512, "mid"))
                tc0_steps += [lambda f=f: ffn1_f(0, 512, f, "dve")
                              for f in range(FC)]
                tc0_steps += [lambda eb=eb: ffn2_eb(0, 512, eb) for eb in range(EC)]
                tcg = gen_of(tc0_steps)
                fillc = {0: [take(tcg, 2)], 1: [take(tcg, 3)], 2: [take(tcg, 3)],
                         3: [take(tcg, 3)], 4: [take(tcg, 3)], 5: [take(tcg, 3)],
                         6: [take(tcg, 4)], 7: [take(tcg, 4)]}
                heads_half(ktp_c, qtp_ct, v_at2, False, "ca", o2, 512, 1024, fillc)
                take(tcg, 99)()

                # cross half1 tail + FFN half1 + transpose/store per eb
                for eb in range(EC):
                    out_proj_eb(o2, wp_ct, bias_sb["bp_c"], "ca", 512, 1024, eb)
                ln_half(xap2, xn3, "ln3b", 512, 1024, "pre")
                for f in range(FC):
                    ffn1_f(512, 1024, f, "act" if f % 2 == 0 else "dve")
                for eb in range(EC):
                    ffn2_eb(512, 1024, eb)
                    for tb in range(R // 128):
                        ps = psum.tile([128, 128], F32, name="trp", tag="o", bufs=2)
                        nc.tensor.transpose(
                            ps, xto[eb][:, tb * 128:(tb + 1) * 128], ident_sb)
                        ott = work.tile([128, 128], F32, name="ott", tag="ott", bufs=4)
                        if tb % 2 == 0:
                            nc.vector.tensor_copy(ott, ps)
                        else:
                            nc.scalar.copy(ott, ps)
                        nc.sync.dma_start(
                            out=out_d[tb * 128:(tb + 1) * 128,
                                      eb * 128:(eb + 1) * 128], in_=ott)

    nc.compile()
    return nc
# ---------------------------------------------------------------------------
# host side
# ---------------------------------------------------------------------------

_CACHE = {}


def _host_prep(inputs, T=T_FULL):
    ii = {k: np.asarray(v, dtype=np.float32) for k, v in inputs.items()}
    g1, be1, g2, be2, g3, be3 = (ii[k] for k in ("g1", "be1", "g2", "be2", "g3", "be3"))

    def fold_qkv(wstk, g, be, scale=1.0):
        wall = np.transpose(wstk, (1, 0, 2)).reshape(E, H * D)  # [E, H*D]
        return ((g[:, None] * wall) * (scale * 16.0)).astype(ml_dtypes.float8_e4m3), \
               ((be @ wall) * scale).astype(np.float32)

    sc = float(D) ** -0.5
    wq_s, bq_s = fold_qkv(ii["Wq_s"], g1, be1, sc)
    wk_s, bk_s = fold_qkv(ii["Wk_s"], g1, be1)
    wv_s, bv_s = fold_qkv(ii["Wv_s"], g1, be1)
    wq_c, bq_c = fold_qkv(ii["Wq_c"], g2, be2, sc)
    wk_c, bk_c = fold_qkv(ii["Wk_c"], np.ones(E, np.float32), np.zeros(E, np.float32))
    wv_c, bv_c = fold_qkv(ii["Wv_c"], np.ones(E, np.float32), np.zeros(E, np.float32))
    assert np.allclose(bv_s, 0, atol=1e-6) and np.allclose(bv_c, 0, atol=1e-6), \
        "V-projection bias folding not implemented (be nonzero)"
    w1 = (g3[:, None] * ii["W1"] * 16.0).astype(ml_dtypes.float8_e4m3)
    b1f = ((be3 @ ii["W1"] + ii["b1"]) * 16.0).astype(np.float32)
    w2 = (ii["W2"] * 16.0).astype(ml_dtypes.float8_e4m3)

    shared = dict(
        wq_s=wq_s, wk_s=wk_s, wv_s=wv_s, wp_s=ii["Wp_s"].astype(ml_dtypes.bfloat16),
        wq_c=wq_c, wk_c=wk_c, wv_c=wv_c, wp_c=ii["Wp_c"].astype(ml_dtypes.bfloat16),
        w1=w1, w2=w2, b1f=b1f,
        bq_s=bq_s, bk_s=bk_s, bq_c=bq_c, bk_c=bk_c,
        bp_s=ii["bp_s"].astype(np.float32), bp_c=ii["bp_c"].astype(np.float32),
        ident=np.eye(128, dtype=np.float32),
    )
    mtri = np.triu(np.ones((128, 128), dtype=np.float32))

    q, k, v = ii["q"], ii["k"], ii["v"]
    n_b = q.shape[0]
    n_stripes = T // 128
    in_maps = []
    for core in range(2 * n_b):
        b, p = core // 2, core % 2
        order = [2 * i + (1 - p) for i in range(n_stripes // 2)] + \
                [2 * i + p for i in range(n_stripes // 2)]
        cols = np.concatenate([np.arange(s * 128, (s + 1) * 128) for s in order])
        m = dict(shared)
        m["qTp"] = np.ascontiguousarray(q[b].T[:, cols])
        m["kT"] = np.ascontiguousarray(k[b].T).astype(ml_dtypes.float8_e4m3)
        m["vT"] = np.ascontiguousarray(v[b].T).astype(ml_dtypes.float8_e4m3)
        m["msk2"] = np.hstack([np.full((128, 128), float(p), np.float32),
                               mtri]).astype(ml_dtypes.bfloat16)
        in_maps.append(m)
    return in_maps


def _gather(results, b2, T=T_FULL, n_b=B):
    out = np.zeros((n_b, T, E), dtype=np.float32)
    for core in range(2 * n_b):
        b, p = core // 2, core % 2
        r = results[core]["out"].reshape(T // 256, 128, E)
        for j in range(T // 256):
            out[b, (2 * j + p) * 128:(2 * j + p + 1) * 128, :] = r[j]
    return out + np.asarray(b2, np.float32)   # b2 bias folded on host


def kernel(**inputs):
    if "nc" not in _CACHE:
        _CACHE["nc"] = build_nc(T_FULL)
    nc = _CACHE["nc"]
    in_maps = _host_prep(inputs, T_FULL)
    res = run_bass_kernel_spmd(nc, in_maps, core_ids=list(range(NCORES)))
    return _gather(res.results, inputs["b2"], T_FULL)



# revision 47
# speedup vs baseline: 1.1003x; 1.0009x over previous
"""Trainium2 Bass kernel for a dense transformer DecoderLayer.

Layer: x = q
  x += SelfAttn(LN1(x))   (causal, 8 heads)
  x += CrossAttn(LN2(x), k, v)
  x += FFN(LN3(x))        (E -> 4E relu -> E)

Sharding: 8 cores = (batch b = core//2, parity p = core%2). Core (b, p)
owns the 8 odd-or-even 128-row stripes of batch b's 2048 query rows.
Host permutes q[b]^T columns to [partner stripes | own stripes] so the
device program is identical on every core (SPMD); the parity-dependent
causal boundary is carried by a data mask (mp = all-ones or all-zeros).

Device dataflow is fully "transposed": the residual stream lives as
x^T [E=512 partitions(4 tiles), tokens] so no on-device activation
transposes are needed except the final 128x128 PE transposes on output.
Scores are computed transposed (S^T [s,q]) so the softmax denominator
falls out of the P@V matmul via an appended ones-column on V.
"""

import numpy as np
import ml_dtypes

import concourse.bass as bass
import concourse.tile as tile
from concourse import bacc
from concourse import mybir
from concourse.bass_utils import run_bass_kernel_spmd

F32 = mybir.dt.float32
F32R = mybir.dt.float32r
BF16 = mybir.dt.bfloat16
FP8 = mybir.dt.float8e4

B, T_FULL, E, H, D, FW = 4, 2048, 512, 8, 64, 4
EC = E // 128           # e-chunks
F = FW * E              # ffn hidden
FC = F // 128
EPS = 1e-5
NCORES = 8


def _pieces(a, b, step=512):
    """Split [a, b) at multiples of `step` (PSUM-bank aligned pieces)."""
    out = []
    while a < b:
        nxt = min(b, (a // step + 1) * step)
        out.append((a, nxt))
        a = nxt
    return out


def _pieces_bf(a, b):
    """Matmul output pieces: one PSUM bank (512 f32) per matmul — walrus
    rejects bank-spanning matmul outputs."""
    return _pieces(a, b, 512)


def build_nc(T=T_FULL):
    R = T // 2           # own query columns (packed at [R:T])
    G = R // 128         # own 128-col groups
    NCH = T // 128       # total s-chunks

    nc = bacc.Bacc(None, target_bir_lowering=False)

    # ---------------- DRAM I/O ----------------
    qTp = nc.dram_tensor("qTp", [E, T], F32, kind="ExternalInput")
    kT = nc.dram_tensor("kT", [E, T], mybir.dt.float8e4, kind="ExternalInput")
    vT = nc.dram_tensor("vT", [E, T], mybir.dt.float8e4, kind="ExternalInput")
    w = {}
    for nm in ("wq_s", "wk_s", "wv_s", "wq_c", "wk_c", "wv_c"):
        w[nm] = nc.dram_tensor(nm, [E, H * D], mybir.dt.float8e4, kind="ExternalInput")
    w["wp_s"] = nc.dram_tensor("wp_s", [H * D, E], BF16, kind="ExternalInput")
    w["wp_c"] = nc.dram_tensor("wp_c", [H * D, E], BF16, kind="ExternalInput")
    w["w1"] = nc.dram_tensor("w1", [E, F], mybir.dt.float8e4, kind="ExternalInput")
    w["w2"] = nc.dram_tensor("w2", [F, E], mybir.dt.float8e4, kind="ExternalInput")
    bias_d = {}
    for nm, sz in (("bq_s", H * D), ("bk_s", H * D), ("bq_c", H * D), ("bk_c", H * D),
                   ("bp_s", E), ("bp_c", E), ("b1f", F)):
        bias_d[nm] = nc.dram_tensor(nm, [sz], F32, kind="ExternalInput")
    msk2_d = nc.dram_tensor("msk2", [128, 256], BF16, kind="ExternalInput")
    ident_d = nc.dram_tensor("ident", [128, 128], F32, kind="ExternalInput")
    out_d = nc.dram_tensor("out", [R, E], F32, kind="ExternalOutput")

    with tile.TileContext(nc) as tc:
        with (
            tc.tile_pool(name="resident", bufs=1) as res,
            tc.tile_pool(name="consts", bufs=1) as cpool,
            tc.tile_pool(name="work", bufs=2) as work,
            tc.tile_pool(name="es_pool", bufs=4) as es_pool,
            tc.tile_pool(name="stat", bufs=2) as stat,
            tc.tile_pool(name="drampool", bufs=2, space="DRAM") as drampool,
            tc.tile_pool(name="psum", bufs=1, space="PSUM") as psum,
        ):
            # ---- PSUM tags: "sc" 2bk x2, "o" 1bk x2, "st" 1bk x2 = 8 banks
            def ps_sc(name):
                """[128, 2, 512] scores-pair tile; projections use [:, 0, :]."""
                return psum.tile([128, 2, 512], F32, name=name, tag="sc", bufs=2)

            def ps_o(name, shape=None):
                return psum.tile(shape or [128, 512], F32, name=name, tag="o", bufs=2)

            # ---------------- resident loads ----------------
            xto = []        # own-half residual stream [E, R], lives whole kernel
            for c in range(EC):
                t_ = res.tile([128, R], F32, name=f"xto{c}")
                nc.sync.dma_start(
                    out=t_, in_=qTp.rearrange("(c p) t -> c p t", p=128)[c][:, R:T])
                xto.append(t_)

            bias_sb = {}
            for nm in bias_d:
                src = bias_d[nm]
                t_ = cpool.tile([128, src.shape[0] // 128], F32, name=f"b_{nm}")
                nc.sync.dma_start(out=t_, in_=src.rearrange("(c p) -> p c", p=128))
                bias_sb[nm] = t_
            msk2_sb = cpool.tile([128, 2, 128], BF16, name="msk2_sb")
            nc.sync.dma_start(out=msk2_sb, in_=msk2_d[:, :])
            ident_sb = cpool.tile([128, 128], F32, name="ident_sb")
            nc.sync.dma_start(out=ident_sb, in_=ident_d[:, :])
            # fp8 ones/E for DoubleRow LN-stats lhsT (middle stride 16: ok)
            ones8 = cpool.tile([128, 2, 16], FP8, name="ones8")
            nc.vector.memset(ones8, 1.0 / E)
            eps_sb = cpool.tile([1, 1], F32, name="eps_sb")
            nc.vector.memset(eps_sb, EPS)

            def load_w(pool, nm, tag):
                src = w[nm]
                if nm.startswith("wp"):
                    # head-pair packed: rows [hp*128:(hp+1)*128] = heads 2hp,2hp+1
                    t_ = pool.tile([128, H // 2, E], BF16, name=f"sb_{nm}", tag=tag)
                    nc.sync.dma_start(out=t_, in_=src.rearrange("(hp p) n -> p hp n", p=128))
                else:
                    t_ = pool.tile([128, src.shape[0] // 128, src.shape[1]],
                                   src.dtype, name=f"sb_{nm}", tag=tag)
                    nc.sync.dma_start(out=t_, in_=src.rearrange("(c p) n -> p c n", p=128))
                return t_

            # ---------------- transposed layernorm (per 512-token piece) ----
            def ln_half(xap, xn8, tag, qa, qb, prof):
                """LN over token cols [qa, qb) (<=512 wide). xap(c, a, b) ->
                [128, b-a] f32 SBUF AP; writes normalized fp8 to
                xn8[:, c, qa:qb]. prof picks engines: "pre" = Act square
                (idle prefix), "mid" = Pool (Act busy with exp)."""
                wd = qb - qa
                xs = work.tile([128, EC, 512], FP8, name=f"xs_{tag}", tag="lnxs", bufs=2)
                sqt = work.tile([128, EC, 512], FP8, name=f"sq_{tag}", tag="lnsq", bufs=2)
                for c in range(EC):
                    xa = xap(c, qa, qb)
                    if prof == "pre":
                        nc.scalar.activation(sqt[:, c, :wd], xa,
                                             mybir.ActivationFunctionType.Square)
                    else:
                        nc.gpsimd.tensor_mul(sqt[:, c, :wd], xa, xa)
                    nc.gpsimd.tensor_copy(xs[:, c, :wd], xa)
                st0 = psum.tile([1, 512], F32, name=f"st0_{tag}", tag="st", bufs=2)
                st1 = psum.tile([1, 512], F32, name=f"st1_{tag}", tag="st", bufs=2)
                for c in range(0, EC, 2):
                    nc.tensor.matmul(st0[:, :wd], ones8[:, :, 0:1], xs[:, c:c + 2, :wd],
                                     start=(c == 0), stop=(c == EC - 2),
                                     perf_mode=mybir.MatmulPerfMode.DoubleRow,
                                     skip_group_check=True)
                    nc.tensor.matmul(st1[:, :wd], ones8[:, :, 0:1], sqt[:, c:c + 2, :wd],
                                     start=(c == 0), stop=(c == EC - 2),
                                     perf_mode=mybir.MatmulPerfMode.DoubleRow,
                                     skip_group_check=True)
                var = stat.tile([1, 512], F32, name=f"var_{tag}", tag="var", bufs=2)
                m_sb = stat.tile([1, 512], F32, name=f"m_{tag}", tag="m_sb", bufs=2)
                nc.vector.tensor_copy(m_sb[:, :wd], st0[:, :wd])
                nc.vector.tensor_mul(var[:, :wd], m_sb[:, :wd], st0[:, :wd])
                nc.vector.tensor_sub(var[:, :wd], st1[:, :wd], var[:, :wd])
                # rsqrt = exp(-0.5*ln(var+eps)): stays in the ln/exp act table
                nc.scalar.activation(var[:, :wd], var[:, :wd],
                                     mybir.ActivationFunctionType.Ln,
                                     bias=eps_sb[0:1, 0:1])
                nc.scalar.activation(var[:, :wd], var[:, :wd],
                                     mybir.ActivationFunctionType.Exp, scale=-0.5)
                mb = work.tile([128, 512], F32, name=f"mb_{tag}", tag="mb", bufs=2)
                rb = work.tile([128, 512], F32, name=f"rb_{tag}", tag="rb", bufs=2)
                m_dr = drampool.tile([1, 512], F32, name=f"mdr_{tag}", tag="mdr", bufs=3)
                r_dr = drampool.tile([1, 512], F32, name=f"rdr_{tag}", tag="rdr", bufs=3)
                nc.sync.dma_start(out=m_dr[:, :wd], in_=m_sb[:, :wd])
                nc.sync.dma_start(out=r_dr[:, :wd], in_=var[:, :wd])
                nc.sync.dma_start(out=mb[:, :wd], in_=m_dr[:, :wd].to_broadcast((128, wd)))
                nc.sync.dma_start(out=rb[:, :wd], in_=r_dr[:, :wd].to_broadcast((128, wd)))
                for c in range(EC):
                    tmp = work.tile([128, 512], F32, name=f"lt_{tag}", tag="lntmp", bufs=2)
                    nc.vector.tensor_sub(tmp[:, :wd], xap(c, qa, qb), mb[:, :wd])
                    nc.vector.tensor_mul(xn8[:, c, qa:qb], tmp[:, :wd], rb[:, :wd])

            # ---------------- attention building blocks ----------------
            def evac_proj(out_ap, ps_ap, b_ap, eng):
                """PSUM -> SBUF evac: out = ps/16 + bias, on chosen engine."""
                if eng == "act":
                    nc.scalar.activation(out_ap, ps_ap,
                                         mybir.ActivationFunctionType.Identity,
                                         bias=b_ap, scale=1.0 / 16.0)
                else:
                    nc.vector.tensor_scalar(out_ap, ps_ap, 1.0 / 16.0, b_ap,
                                            op0=mybir.AluOpType.mult,
                                            op1=mybir.AluOpType.add)

            def proj_kt(apool, tags, src_aps, wk_t, bk_sb, tag, order=None):
                """K^T head-pair tiles [128, T]; emits all pieces (prefix)."""
                ktp = [apool.tile([128, T], BF16, name=f"ktp_{tag}{pp}", tag=tags[pp])
                       for pp in range(4)]
                pcs = order or _pieces_bf(0, T)
                for (a, b_) in pcs:
                    for pp in range(4):
                        ps = ps_sc(f"kps_{tag}")
                        for ci, c in enumerate(range(0, EC, 2)):
                            nc.tensor.matmul(
                                ps[:, 0, :b_ - a],
                                wk_t[:, c:c + 2, pp * 128:(pp + 1) * 128],
                                src_aps[ci](a, b_),
                                start=(c == 0), stop=(c == EC - 2),
                                perf_mode=mybir.MatmulPerfMode.DoubleRow)
                        evac_proj(ktp[pp][:, a:b_], ps[:, 0, :b_ - a],
                                  bk_sb[:, pp:pp + 1], "act" if pp % 2 == 0 else "dve")
                return ktp

            def proj_qt_half(qtp, xn_q, wq_t, bq_sb, tag, qa, qb, eng, pp=None):
                """Q^T projection for token cols [qa, qb). pp=None -> all 4."""
                for p_ in ([pp] if pp is not None else range(4)):
                    ps = ps_sc(f"qps_{tag}")
                    for ci, c in enumerate(range(0, EC, 2)):
                        nc.tensor.matmul(
                            ps[:, 0, :qb - qa],
                            wq_t[:, c:c + 2, p_ * 128:(p_ + 1) * 128],
                            xn_q[ci](qa, qb),
                            start=(c == 0), stop=(c == EC - 2),
                            perf_mode=mybir.MatmulPerfMode.DoubleRow)
                    e_ = ("act" if p_ % 2 == 0 else "dve") if eng == "mix0" else eng
                    evac_proj(qtp[p_][:, qa:qb], ps[:, 0, :qb - qa],
                              bq_sb[:, p_:p_ + 1], e_)

            def heads_half(ktp, qtp, v_pair, causal, tag, o_out, qa, qb,
                           fillers=None, heads=None):
                """One q-half (cols [qa, qb)) of 8 heads. v_pair(pr, h) ->
                lhsT AP [128, 2, D+2] fp8. Scores for a chunk pair share one
                [128, 2, 512] PSUM tile -> one exp -> fp8 es -> DoubleRow P@V.
                Scores/exp are emitted LAG pairs ahead of P@V so a P@V stalled
                on the o-buffer rotation cannot block upcoming scores (and so
                starve the Act engine). fillers[h]: callables emitted after
                head h's last scores."""
                VP = D + 2
                wd0 = qb - qa
                sc_steps, pv_steps = [], []
                for h in (heads if heads is not None else range(H)):
                    pp, hr = h // 2, (h % 2) * 64
                    prs = [g for g in range(G) if (not causal) or g * 128 < qb]
                    o_cell = {}

                    def mk_sc(h=h, pp=pp, hr=hr, cell=None):
                        def run():
                            pr, es_cell = cell["pr"], cell
                            if causal:
                                kA, kB, q0 = pr, G + pr, pr * 128
                            else:
                                kA, kB, q0 = 2 * pr, 2 * pr + 1, 0
                            a = max(q0, qa)
                            wd = qb - a
                            sc = ps_sc(f"scps_{tag}")
                            nc.tensor.matmul(
                                sc[:, 0, :wd],
                                ktp[pp][hr:hr + 64, kA * 128:(kA + 1) * 128],
                                qtp[pp][hr:hr + 64, a:qb],
                                start=True, stop=True)
                            nc.tensor.matmul(
                                sc[:, 1, :wd],
                                ktp[pp][hr:hr + 64, kB * 128:(kB + 1) * 128],
                                qtp[pp][hr:hr + 64, a:qb],
                                start=True, stop=True)
                            es = es_pool.tile([128, 2, 512], FP8,
                                              name=f"es_{tag}", tag="es")
                            nc.scalar.activation(es[:, :, :wd], sc[:, :, :wd],
                                                 mybir.ActivationFunctionType.Exp)
                            if causal and q0 >= qa:
                                nc.gpsimd.tensor_mul(es[:, :, 0:128],
                                                     es[:, :, 0:128], msk2_sb)
                            es_cell["es"], es_cell["a"], es_cell["wd"] = es, a, wd
                        return run

                    def mk_pv(h=h, pp=pp, hr=hr, o_cell=o_cell, cell=None,
                              first=False, last=False):
                        def run():
                            pr, es, a, wd = cell["pr"], cell["es"], cell["a"], cell["wd"]
                            if first:
                                o_cell["ps"] = ps_o(f"ops_{tag}", [VP, 512])
                            o_ps = o_cell["ps"]
                            nc.tensor.matmul(
                                o_ps[:, a - qa:wd0], v_pair(pr, h), es[:, :, :wd],
                                start=first, stop=last,
                                perf_mode=mybir.MatmulPerfMode.DoubleRow,
                                skip_group_check=True)
                            if last:
                                dn = stat.tile([1, 512], F32, name=f"dn_{tag}", tag="dn")
                                nc.vector.reciprocal(dn[:, :wd0], o_ps[D:D + 1, :wd0])
                                rb_h = work.tile([64, 512], F32, name=f"rbh_{tag}",
                                                 tag="rbh", bufs=2)
                                nc.gpsimd.partition_broadcast(rb_h[:, :wd0], dn[:, :wd0])
                                nc.vector.tensor_mul(o_out[pp][hr:hr + 64, qa:qb],
                                                     o_ps[0:D, :wd0], rb_h[:, :wd0])
                        return run

                    for pi, pr in enumerate(prs):
                        cell = {"pr": pr}
                        sc_steps.append(mk_sc(cell=cell))
                        pv_steps.append(mk_pv(cell=cell, first=(pi == 0),
                                              last=(pi == len(prs) - 1)))
                    sc_steps.append(("fill", h))
                    pv_steps.append(None)

                LAG = 3
                si = pi_ = 0
                emitted_pv = 0
                flat_sc = sc_steps
                # walk: emit sc steps; pv steps trail LAG real-sc-steps behind
                real_idx = []
                for i, s_ in enumerate(flat_sc):
                    if not isinstance(s_, tuple):
                        real_idx.append(i)
                pv_of = {}   # sc position -> pv closure
                k = 0
                for i, s_ in enumerate(flat_sc):
                    if not isinstance(s_, tuple):
                        pv_of[i] = pv_steps[k]
                    k += 1 if True else 0
                # simpler: rebuild aligned lists
                sc_only = [s_ for s_ in flat_sc if not isinstance(s_, tuple)]
                pv_only = [p_ for p_ in pv_steps if p_ is not None]
                fill_pos = {}   # index into sc_only after which filler h runs
                cnt = 0
                for s_ in flat_sc:
                    if isinstance(s_, tuple):
                        fill_pos[cnt - 1] = s_[1]
                    else:
                        cnt += 1
                n = len(sc_only)
                for i in range(n + LAG):
                    if i < n:
                        sc_only[i]()
                    if i - LAG >= 0:
                        pv_only[i - LAG]()
                    if i < n and i in fill_pos and fillers:
                        for f_ in fillers.get(fill_pos[i], []):
                            f_()

            def out_proj_eb(o_pairs, wp_t, bp_sb, tag, qa, qb, eb):
                ps = ps_o(f"yps_{tag}")
                for j in range(H // 2):
                    nc.tensor.matmul(
                        ps[:, :qb - qa],
                        wp_t[:, j, eb * 128:(eb + 1) * 128],
                        o_pairs[j][:, qa:qb],
                        start=(j == 0), stop=(j == H // 2 - 1))
                nc.vector.scalar_tensor_tensor(
                    xto[eb][:, qa:qb], ps[:, :qb - qa], bp_sb[:, eb:eb + 1],
                    xto[eb][:, qa:qb],
                    op0=mybir.AluOpType.add, op1=mybir.AluOpType.add)

            # ================ forward ================
            with tc.tile_pool(name="w_attn", bufs=1) as wat, \
                 tc.tile_pool(name="p_self", bufs=1) as pself:
                # partner-half of residual input (dies after LN1)
                xtp = []
                for c in range(EC):
                    t_ = pself.tile([128, R], F32, name=f"xtp{c}", tag=f"xtp{c}")
                    nc.sync.dma_start(
                        out=t_, in_=qTp.rearrange("(c p) t -> c p t", p=128)[c][:, 0:R])
                    xtp.append(t_)

                wq_t = load_w(wat, "wq_s", "wq")
                wk_t = load_w(wat, "wk_s", "wk")
                wv_t = load_w(wat, "wv_s", "wv")

                def xap1(c, a, b_):
                    if b_ <= R:
                        return xtp[c][:, a:b_]
                    return xto[c][:, a - R:b_ - R]

                # LN1: emit only the pieces heads-h0 needs, defer the rest
                xn1 = pself.tile([128, EC, T], FP8, name="xn1")
                ln_half(xap1, xn1, "ln1", 0, 512, "pre")
                ln_half(xap1, xn1, "ln1", 1024, 1536, "pre")

                # --- self-attn projections: first-need pieces, rest deferred
                ktp_s = [pself.tile([128, T], BF16, name=f"ktp_s{pp}",
                                    tag=f"ktp{pp}") for pp in range(4)]
                xn_k1 = [lambda a, b_, c=c: xn1[:, c:c + 2, a:b_]
                         for c in range(0, EC, 2)]

                def k_step_s(a, b_, pp, eng="dve"):
                    def run():
                        ps = ps_sc("kps_sa")
                        for ci, c in enumerate(range(0, EC, 2)):
                            nc.tensor.matmul(
                                ps[:, 0, :b_ - a],
                                wk_t[:, c:c + 2, pp * 128:(pp + 1) * 128],
                                xn_k1[ci](a, b_),
                                start=(c == 0), stop=(c == EC - 2),
                                perf_mode=mybir.MatmulPerfMode.DoubleRow)
                        evac_proj(ktp_s[pp][:, a:b_], ps[:, 0, :b_ - a],
                                  bias_sb["bk_s"][:, pp:pp + 1], eng)
                    return run

                for (a, b_) in ((0, 512), (1024, 1536)):
                    for pp in range(4):
                        k_step_s(a, b_, pp, "act" if pp % 2 == 0 else "dve")()
                qtp_s = [pself.tile([128, R], BF16, name=f"qtp_s{pp}", tag=f"qtp{pp}")
                         for pp in range(4)]
                xn_q1 = [lambda a, b_, c=c: xn1[:, c:c + 2, R + a:R + b_]
                         for c in range(0, EC, 2)]
                proj_qt_half(qtp_s, xn_q1, wq_t, bias_sb["bq_s"], "sa", 0, 512, "mix0")
                # V chunk-pair tiles [128, 2, H, D+2] fp8: slot 0 = partner
                # stripe g (chunk g), slot 1 = own stripe g (chunk G+g)
                v_sb_s = [pself.tile([128, 2, H, D + 2], FP8, name=f"vpr_sa{g}",
                                     tag=f"vsb{g}") for g in range(G)]

                def v_step_s(k, eng):
                    def run():
                        pair, slot = (k, 0) if k < G else (k - G, 1)
                        ps = ps_sc("vps_sa")
                        for c in range(0, EC, 2):
                            nc.tensor.matmul(
                                ps[:, 0, :512],
                                xn1[:, c:c + 2, k * 128:(k + 1) * 128],
                                wv_t[:, c:c + 2, :],
                                start=(c == 0), stop=(c == EC - 2),
                                perf_mode=mybir.MatmulPerfMode.DoubleRow)
                        vs = v_sb_s[pair]
                        if eng == "act":
                            nc.scalar.activation(
                                vs[:, slot, :, 0:D],
                                ps[:, 0, :512].rearrange("p (h d) -> p h d", h=H),
                                mybir.ActivationFunctionType.Copy, scale=1.0 / 16.0)
                        else:
                            nc.vector.tensor_scalar(
                                vs[:, slot, :, 0:D],
                                ps[:, 0, :512].rearrange("p (h d) -> p h d", h=H),
                                1.0 / 16.0, None, op0=mybir.AluOpType.mult)
                        nc.gpsimd.memset(vs[:, slot, :, D:D + 2], 1.0)
                    return run

                for k in list(range(0, 4)) + list(range(8, 12)):
                    v_step_s(k, "act" if k % 2 == 0 else "dve")()

                # cross/FFN weights: DMA-queued here so they don't delay
                # the LN1 broadcasts, but still arrive well before first use
                wp_t = load_w(wat, "wp_s", "wp")
                wk_ct = load_w(wat, "wk_c", "wk_c")
                wv_ct = load_w(wat, "wv_c", "wv_c")
                wq_ct = load_w(wat, "wq_c", "wq_c")
                wp_ct = load_w(wat, "wp_c", "wp_c")
                w1_t = load_w(wat, "w1", "w1")
                w2_t = load_w(wat, "w2", "w2")

                # --- cross K/V projection steps (fillers) ---
                ktp_c = [pself.tile([128, T], BF16, name=f"ktp_c{pp}",
                                    tag=f"xn1{'abcd'[pp]}") for pp in range(4)]
                kcache = {}

                def ck_step(a, b_, pp):
                    def run():
                        if a not in kcache:
                            t_ = pself.tile([128, EC, 512], FP8, name="ksl_ca",
                                            tag=f"xtp{(a // 512) % 4}", bufs=1)
                            nc.sync.dma_start(
                                out=t_[:, :, :b_ - a],
                                in_=kT.rearrange("(c p) t -> p c t", p=128)[:, :, a:b_])
                            kcache[a] = t_
                        ps = ps_sc("kps_ca")
                        for c in range(0, EC, 2):
                            nc.tensor.matmul(
                                ps[:, 0, :b_ - a],
                                wk_ct[:, c:c + 2, pp * 128:(pp + 1) * 128],
                                kcache[a][:, c:c + 2, :b_ - a],
                                start=(c == 0), stop=(c == EC - 2),
                                perf_mode=mybir.MatmulPerfMode.DoubleRow)
                        evac_proj(ktp_c[pp][:, a:b_], ps[:, 0, :b_ - a],
                                  bias_sb["bk_c"][:, pp:pp + 1], "dve")
                    return run

                v_g = [pself.tile([128, 8, H, D + 2], FP8, name=f"vg{g2}",
                                  tag=f"vg{g2}") for g2 in range(2)]
                vcache = {}

                def cv_step(k):
                    def run():
                        k4 = (k // 4) * 4
                        if k4 not in vcache:
                            vsl = work.tile([128, EC, 512], FP8, name="vsl_ca",
                                            tag="vsl", bufs=2)
                            nc.sync.dma_start(
                                out=vsl,
                                in_=vT.rearrange("(c p) t -> p c t", p=128)
                                [:, :, k4 * 128:(k4 + 4) * 128])
                            vcache[k4] = vsl
                        off = (k - k4) * 128
                        ps = ps_sc("vps_ca")
                        for c in range(0, EC, 2):
                            nc.tensor.matmul(
                                ps[:, 0, :512],
                                vcache[k4][:, c:c + 2, off:off + 128],
                                wv_ct[:, c:c + 2, :],
                                start=(c == 0), stop=(c == EC - 2),
                                perf_mode=mybir.MatmulPerfMode.DoubleRow)
                        nc.vector.tensor_scalar(
                            v_g[k // 8][:, k % 8, :, 0:D],
                            ps[:, 0, :512].rearrange("p (h d) -> p h d", h=H),
                            1.0 / 16.0, None, op0=mybir.AluOpType.mult)
                        nc.gpsimd.memset(v_g[k // 8][:, k % 8, :, D:D + 2], 1.0)
                    return run

                def gen_of(steps):
                    for s_ in steps:
                        s_()
                        yield

                def take(gen, n):
                    def run():
                        for _ in range(n):
                            try:
                                next(gen)
                            except StopIteration:
                                break
                    return run

                # deferred prefix work, ordered so heads-h1 inputs finish first
                rest_steps = [lambda: ln_half(xap1, xn1, "ln1", 512, 1024, "mid")]
                rest_steps += [k_step_s(512, 1024, pp) for pp in range(4)]
                rest_steps.append(lambda: ln_half(xap1, xn1, "ln1", 1536, 2048, "mid"))
                rest_steps += [k_step_s(1536, 2048, pp) for pp in range(4)]
                rest_steps += [lambda pp=pp: proj_qt_half(
                    qtp_s, xn_q1, wq_t, bias_sb["bq_s"], "sa", 512, 1024, "dve", pp)
                    for pp in range(4)]
                rest_steps += [v_step_s(k, "dve")
                               for k in list(range(4, 8)) + list(range(12, 16))]
                rsg = gen_of(rest_steps)
                ckg = gen_of([ck_step(a, b_, pp)
                              for (a, b_) in _pieces_bf(0, T) for pp in range(4)])
                cvg = gen_of([cv_step(k) for k in range(NCH)])

                # --- self attention, q-half pipelined ---
                o1 = [pself.tile([128, R], BF16, name=f"opr1{j}", tag=f"opr{j}")
                      for j in range(H // 2)]
                v_at1 = lambda pr, h: v_sb_s[pr][:, :, h, :]
                fill0 = {0: [take(rsg, 1)], 1: [take(rsg, 4)], 2: [take(rsg, 5)],
                         3: [take(rsg, 4)], 4: [take(rsg, 4)], 5: [take(rsg, 4)],
                         6: [take(rsg, 2), take(ckg, 2)],
                         7: [take(ckg, 3), take(cvg, 3)]}
                heads_half(ktp_s, qtp_s, v_at1, True, "sa", o1, 0, 512, fill0)
                take(rsg, 99)()

                # tail of half0 (outproj + LN2 + Qc) woven into half1
                xn2 = pself.tile([128, EC, R], FP8, name="xn2")
                qtp_ct = [pself.tile([128, R], BF16, name=f"qtp_c{pp}",
                                     tag=f"qtpc{pp}") for pp in range(4)]
                xap2 = lambda c, a, b_: xto[c][:, a:b_]
                xn_q2 = [lambda a, b_, c=c: xn2[:, c:c + 2, a:b_]
                         for c in range(0, EC, 2)]
                t0_steps = [lambda eb=eb: out_proj_eb(o1, wp_t, bias_sb["bp_s"],
                                                      "sa", 0, 512, eb)
                            for eb in range(EC)]
                t0_steps.append(lambda: ln_half(xap2, xn2, "ln2a", 0, 512, "mid"))
                t0_steps += [lambda pp=pp: proj_qt_half(
                    qtp_ct, xn_q2, wq_ct, bias_sb["bq_c"], "ca", 0, 512, "dve", pp)
                    for pp in range(4)]
                t0g = gen_of(t0_steps)
                # o2 aliases qtp_s tags: qtp_s[j] is last read by self-h1 head
                # 2j+1, well before cross-h0 head 2j writes o2[j]
                o2 = [pself.tile([128, R], BF16, name=f"opr2{j}", tag=f"qtp{j}")
                      for j in range(H // 2)]
                v_at2 = lambda pr, h: v_g[(2 * pr) // 8][:, (2 * pr) % 8:(2 * pr) % 8 + 2, h, :]

                def xh0(heads):
                    return lambda: heads_half(ktp_c, qtp_ct, v_at2, False, "ca",
                                              o2, 0, 512, None, heads=heads)

                fill1 = {0: [take(ckg, 6), take(cvg, 3)],
                         1: [take(ckg, 5), take(cvg, 3)],
                         2: [take(cvg, 3), take(t0g, 2)],
                         3: [take(cvg, 4), take(t0g, 2)],
                         4: [take(t0g, 1)], 5: [take(t0g, 2)],
                         6: [take(t0g, 2), xh0([0, 1])],
                         7: [xh0([2, 3])]}
                heads_half(ktp_s, qtp_s, v_at1, True, "sa", o1, 512, 1024, fill1)
                take(ckg, 99)(); take(cvg, 99)(); take(t0g, 99)()

                # half1 tail: out-proj, then rest of cross-h0 with LN2-h1/Qc-h1
                for eb in range(EC):
                    out_proj_eb(o1, wp_t, bias_sb["bp_s"], "sa", 512, 1024, eb)
                tb_steps = [lambda: ln_half(xap2, xn2, "ln2b", 512, 1024, "mid")]
                tb_steps += [lambda pp=pp: proj_qt_half(
                    qtp_ct, xn_q2, wq_ct, bias_sb["bq_c"], "ca", 512, 1024, "dve", pp)
                    for pp in range(4)]
                tbg = gen_of(tb_steps)
                fillc0 = {4: [take(tbg, 1)], 5: [take(tbg, 2)], 6: [take(tbg, 2)]}
                heads_half(ktp_c, qtp_ct, v_at2, False, "ca", o2, 0, 512, fillc0,
                           heads=[4, 5, 6, 7])
                take(tbg, 99)()

                # tail of cross-h0 (outproj + LN3 + FFN-h0) woven into cross-h1
                # tag-alias onto tiles whose last readers are already done:
                # xn3 reuses xn2's space (dead after qtp_c), h1t reuses xn1's
                xn3 = pself.tile([128, EC, R], FP8, name="xn3", tag="xn2")
                h1t = pself.tile([128, FC, R], FP8, name="h1t", tag="xn1")

                def ffn1_f(qa, qb, f, eng):
                    # h0 fillers use the stats bank; the h1 tail can take the
                    # scores tag (attention is finished there)
                    if qa == 0:
                        ps = psum.tile([128, 512], F32, name="hps", tag="st", bufs=2)
                    else:
                        ps = ps_sc("hps")[:, 0, :]
                    for c in range(0, EC, 2):
                        nc.tensor.matmul(
                            ps[:, :qb - qa],
                            w1_t[:, c:c + 2, f * 128:(f + 1) * 128],
                            xn3[:, c:c + 2, qa:qb],
                            start=(c == 0), stop=(c == EC - 2),
                            perf_mode=mybir.MatmulPerfMode.DoubleRow)
                    if eng == "act":
                        nc.scalar.activation(
                            h1t[:, f, qa:qb], ps[:, :qb - qa],
                            mybir.ActivationFunctionType.Relu,
                            bias=bias_sb["b1f"][:, f:f + 1])
                    else:
                        nc.vector.tensor_scalar(
                            h1t[:, f, qa:qb], ps[:, :qb - qa],
                            bias_sb["b1f"][:, f:f + 1], 0.0,
                            op0=mybir.AluOpType.add, op1=mybir.AluOpType.max)

                def ffn2_eb(qa, qb, eb):
                    ps = ps_o("y2ps")
                    for f in range(0, FC, 2):
                        nc.tensor.matmul(
                            ps[:, :qb - qa],
                            w2_t[:, f:f + 2, eb * 128:(eb + 1) * 128],
                            h1t[:, f:f + 2, qa:qb],
                            start=(f == 0), stop=(f == FC - 2),
                            perf_mode=mybir.MatmulPerfMode.DoubleRow)
                    nc.vector.scalar_tensor_tensor(
                        xto[eb][:, qa:qb], ps[:, :qb - qa], 1.0 / 256.0,
                        xto[eb][:, qa:qb],
                        op0=mybir.AluOpType.mult, op1=mybir.AluOpType.add)

                tc0_steps = [lambda eb=eb: out_proj_eb(o2, wp_ct, bias_sb["bp_c"],
                                                       "ca", 0, 512, eb)
                             for eb in range(EC)]
                tc0_steps.append(lambda: ln_half(xap2, xn3, "ln3a", 0, 512, "mid"))
                tc0_steps += [lambda f=f: ffn1_f(0, 512, f, "dve")
                              for f in range(FC)]
                tc0_steps += [lambda eb=eb: ffn2_eb(0, 512, eb) for eb in range(EC)]
                tcg = gen_of(tc0_steps)
                fillc = {0: [take(tcg, 2)], 1: [take(tcg, 3)], 2: [take(tcg, 3)],
                         3: [take(tcg, 3)], 4: [take(tcg, 3)], 5: [take(tcg, 3)],
                         6: [take(tcg, 4)], 7: [take(tcg, 4)]}
                heads_half(ktp_c, qtp_ct, v_at2, False, "ca", o2, 512, 1024, fillc)
                take(tcg, 99)()

                # cross half1 tail + FFN half1 + transpose/store per eb
                for eb in range(EC):
                    out_proj_eb(o2, wp_ct, bias_sb["bp_c"], "ca", 512, 1024, eb)
                ln_half(xap2, xn3, "ln3b", 512, 1024, "pre")
                for f in range(FC):
                    ffn1_f(512, 1024, f, "act" if f % 2 == 0 else "dve")
                for eb in range(EC):
                    ffn2_eb(512, 1024, eb)
                    for tb in range(R // 128):
                        ps = psum.tile([128, 128], F32, name="trp", tag="o", bufs=2)
                        nc.tensor.transpose(
                            ps, xto[eb][:, tb * 128:(tb + 1) * 128], ident_sb)
                        ott = work.tile([128, 128], F32, name="ott", tag="ott", bufs=4)
                        if tb % 2 == 0:
                            nc.vector.tensor_copy(ott, ps)
                        else:
                            nc.scalar.copy(ott, ps)
                        nc.sync.dma_start(
                            out=out_d[tb * 128:(tb + 1) * 128,
                                      eb * 128:(eb + 1) * 128], in_=ott)

    nc.compile()
    return nc
# ---------------------------------------------------------------------------
# host side
# ---------------------------------------------------------------------------

_CACHE = {}


def _host_prep(inputs, T=T_FULL):
    ii = {k: np.asarray(v, dtype=np.float32) for k, v in inputs.items()}
    g1, be1, g2, be2, g3, be3 = (ii[k] for k in ("g1", "be1", "g2", "be2", "g3", "be3"))

    def fold_qkv(wstk, g, be, scale=1.0):
        wall = np.transpose(wstk, (1, 0, 2)).reshape(E, H * D)  # [E, H*D]
        return ((g[:, None] * wall) * (scale * 16.0)).astype(ml_dtypes.float8_e4m3), \
               ((be @ wall) * scale).astype(np.float32)

    sc = float(D) ** -0.5
    wq_s, bq_s = fold_qkv(ii["Wq_s"], g1, be1, sc)
    wk_s, bk_s = fold_qkv(ii["Wk_s"], g1, be1)
    wv_s, bv_s = fold_qkv(ii["Wv_s"], g1, be1)
    wq_c, bq_c = fold_qkv(ii["Wq_c"], g2, be2, sc)
    wk_c, bk_c = fold_qkv(ii["Wk_c"], np.ones(E, np.float32), np.zeros(E, np.float32))
    wv_c, bv_c = fold_qkv(ii["Wv_c"], np.ones(E, np.float32), np.zeros(E, np.float32))
    assert np.allclose(bv_s, 0, atol=1e-6) and np.allclose(bv_c, 0, atol=1e-6), \
        "V-projection bias folding not implemented (be nonzero)"
    w1 = (g3[:, None] * ii["W1"] * 16.0).astype(ml_dtypes.float8_e4m3)
    b1f = ((be3 @ ii["W1"] + ii["b1"]) * 16.0).astype(np.float32)
    w2 = (ii["W2"] * 16.0).astype(ml_dtypes.float8_e4m3)

    shared = dict(
        wq_s=wq_s, wk_s=wk_s, wv_s=wv_s, wp_s=ii["Wp_s"].astype(ml_dtypes.bfloat16),
        wq_c=wq_c, wk_c=wk_c, wv_c=wv_c, wp_c=ii["Wp_c"].astype(ml_dtypes.bfloat16),
        w1=w1, w2=w2, b1f=b1f,
        bq_s=bq_s, bk_s=bk_s, bq_c=bq_c, bk_c=bk_c,
        bp_s=ii["bp_s"].astype(np.float32), bp_c=ii["bp_c"].astype(np.float32),
        ident=np.eye(128, dtype=np.float32),
    )
    mtri = np.triu(np.ones((128, 128), dtype=np.float32))

    q, k, v = ii["q"], ii["k"], ii["v"]
    n_b = q.shape[0]
    n_stripes = T // 128
    in_maps = []
    for core in range(2 * n_b):
        b, p = core // 2, core % 2
        order = [2 * i + (1 - p) for i in range(n_stripes // 2)] + \
                [2 * i + p for i in range(n_stripes // 2)]
        cols = np.concatenate([np.arange(s * 128, (s + 1) * 128) for s in order])
        m = dict(shared)
        m["qTp"] = np.ascontiguousarray(q[b].T[:, cols])
        m["kT"] = np.ascontiguousarray(k[b].T).astype(ml_dtypes.float8_e4m3)
        m["vT"] = np.ascontiguousarray(v[b].T).astype(ml_dtypes.float8_e4m3)
        m["msk2"] = np.hstack([np.full((128, 128), float(p), np.float32),
                               mtri]).astype(ml_dtypes.bfloat16)
        in_maps.append(m)
    return in_maps


def _gather(results, b2, T=T_FULL, n_b=B):
    out = np.zeros((n_b, T, E), dtype=np.float32)
    for core in range(2 * n_b):
        b, p = core // 2, core % 2
        r = results[core]["out"].reshape(T // 256, 128, E)
        for j in range(T // 256):
            out[b, (2 * j + p) * 128:(2 * j + p + 1) * 128, :] = r[j]
    return out + np.asarray(b2, np.float32)   # b2 bias folded on host


def kernel(**inputs):
    if "nc" not in _CACHE:
        _CACHE["nc"] = build_nc(T_FULL)
    nc = _CACHE["nc"]
    in_maps = _host_prep(inputs, T_FULL)
    res = run_bass_kernel_spmd(nc, in_maps, core_ids=list(range(NCORES)))
    return _gather(res.results, inputs["b2"], T_FULL)

